# revision 7
# baseline (speedup 1.0000x reference)
"""Bass/Trainium2 kernel for nn_EuclideanGraphEncoder (GCN message passing).

Strategy: data-parallel over the batch (4 graphs per core, 8 cores),
weights replicated, no collectives. The dominant aggregation matmul
(adj @ msg) runs in fp8-e4m3 DoubleRow mode (2 k-tiles per instruction,
2x fp8 throughput): the adjacency is downcast+transposed to fp8 on the
host, and the per-layer messages are quantized to fp8 on the DVE during
the PSUM->SBUF bias-add. Node features h stay fp16 (their quantization
error would hit the output un-averaged), so the final rel-err keeps the
adjacency/message fp8 noise only, which the 1024-wide sum-aggregation
averages down to ~1.5e-2.

Device-side layout: h is kept transposed [hid=128 partitions, n=1024]
fp16. Per layer:
  msg[n,k]  = h @ Wl          (8x K=128 matmuls; DVE adds bias, casts fp8)
  aggT[k,n] = msg.T @ adjT    (2 n-tiles x 4 DoubleRow fp8 matmuls)
  hT        = relu(aggT)      (ACT, PSUM->SBUF fp16, power-of-2 rescale)
Projection keeps the transposed layout (W_proj stationary, hT moving,
N=512) and returns outT [64, n] fp16 scaled by 1/S3; the host transposes,
rescales, adds b_proj and applies the node mask.

Scheduling: dependency-free warm matmuls spin the PE through the HAM
clock ramp while the DMA rings spin up; the per-layer biases arrive as
[1, 512] rows and are partition-broadcast by a rank-1 f32r matmul
(saves 3x256KB of head-of-queue DMA); graphs run in software-pipelined
pairs so each graph's relu -> msg -> fp8-quantize chain hides under the
other graph's DoubleRow aggregations; out stores trigger from SWDGE so
the ACT engine never stalls on descriptor generation.
"""

import sys
from contextlib import ExitStack

import ml_dtypes
import numpy as np

try:
    import concourse.bass as bass
except ImportError:  # fall back to the repo checkout
    sys.path.insert(0, "/opt/trn_rl_repo")
    import concourse.bass as bass

import concourse.tile as tile
from concourse import bacc, mybir
from concourse.bass_utils import run_bass_kernel_spmd

B, N, IN_DIM, HID, OUT = 32, 1024, 64, 128, 64
NUM_LAYERS = 3
N_CORES = 8
BPC = B // N_CORES  # graphs per core
NT = N // 512  # aggregation free-dim tiles
NC8 = N // 128  # node chunks of 128

FP8 = mybir.dt.float8e4
FP16 = mybir.dt.float16
FP32 = mybir.dt.float32
FP32R = mybir.dt.float32r
RELU = mybir.ActivationFunctionType.Relu
DR = mybir.MatmulPerfMode.DoubleRow

# Per-layer power-of-2 scales: SBUF h/msg tiles hold h_true / S[i] so fp16
# never overflows (true agg magnitudes reach ~5e6). Scale hops are exact
# (powers of two) and ride the relu ACT scale; the host pre-scales the
# biases and post-scales the output.
S = [1.0, 64.0, 16384.0, 4194304.0]

WARM_MMS = 30  # PE clock warm-up matmuls covering the DMA-ring spin-up


def _kernel_body(ctx, tc, out, adjT8, xT, w_embed, wl, blrT, w_proj):
    nc = tc.nc

    consts = ctx.enter_context(tc.tile_pool(name="consts", bufs=1))
    adj_pool = ctx.enter_context(tc.tile_pool(name="adj", bufs=BPC * NT))
    xt_pool = ctx.enter_context(tc.tile_pool(name="xt", bufs=BPC))
    h_pool = ctx.enter_context(tc.tile_pool(name="h", bufs=8))
    msg_pool = ctx.enter_context(tc.tile_pool(name="msg", bufs=6))
    o_pool = ctx.enter_context(tc.tile_pool(name="o", bufs=BPC))
    psA = ctx.enter_context(tc.tile_pool(name="psA", bufs=3, space="PSUM"))
    psM = ctx.enter_context(tc.tile_pool(name="psM", bufs=3, space="PSUM"))
    psO = ctx.enter_context(tc.tile_pool(name="psO", bufs=2, space="PSUM"))

    # ---- PE clock pre-warm -------------------------------------------
    # Dependency-free matmuls from t=0: the HAM un-throttles (1.2 ->
    # 2.4 GHz) after ~3.4us of sustained PE activity, so the real work
    # (gated on the first DMA deliveries at ~7us) starts at full clock.
    warm_w = consts.tile([1, HID], FP16, tag="warm_w")
    warm_m = consts.tile([1, 512], FP16, tag="warm_m")
    nc.vector.memset(warm_w[:], 0.0)
    nc.vector.memset(warm_m[:], 0.0)
    ones_h = consts.tile([1, HID], FP16, tag="ones_h")
    nc.vector.memset(ones_h[:], 1.0)
    for w in range(WARM_MMS):
        psw = psA.tile([HID, 512], FP32, tag="psA", name="psw")
        nc.tensor.matmul(psw[:], warm_w[:], warm_m[:], start=True, stop=True)

    # ---- loads --------------------------------------------------------
    # Queue FIFO order == delivery order. Sync HWDGE ring: the tensors
    # gating graph-pair (0,1)'s prologue, then the adj flood. The second
    # HWDGE ring (scalar) carries x1 in parallel; everything needed later
    # rides SWDGE (gpsimd), which also triggers the out stores.
    xts = [xt_pool.tile([IN_DIM, N], FP16, tag="xt", name=f"xt{bb}")
           for bb in range(BPC)]
    we_t = consts.tile([IN_DIM, HID], FP16, tag="we")
    wl_t = [consts.tile([HID, HID], FP16, tag=f"wl{i}", name=f"wl{i}")
            for i in range(NUM_LAYERS)]
    blr_t = [consts.tile([1, 4 * HID], FP16, tag=f"blr{i}", name=f"blr{i}")
             for i in range(NUM_LAYERS)]
    bl_t = [consts.tile([128, 4 * HID], FP32, tag=f"bl{i}", name=f"bl{i}")
            for i in range(NUM_LAYERS)]
    wp_t = consts.tile([HID, OUT], FP16, tag="wp")

    nc.sync.dma_start(xts[0][:], xT[0])
    nc.sync.dma_start(we_t[:], w_embed[:, :])
    nc.sync.dma_start(wl_t[0][:], wl[0])
    for i in range(NUM_LAYERS):
        nc.sync.dma_start(blr_t[i][:], blrT[i])
    nc.scalar.dma_start(xts[1][:], xT[1])
    for bb in (2, 3):
        nc.gpsimd.dma_start(xts[bb][:], xT[bb])
    for i in range(1, NUM_LAYERS):
        nc.gpsimd.dma_start(wl_t[i][:], wl[i])
    nc.gpsimd.dma_start(wp_t[:], w_proj[:, :])

    # adj: one 512KB DMA per (graph, n-half); 4KB contiguous per partition.
    adj_t = [[adj_pool.tile([128, NC8, 512], FP8, tag="adj",
                            name=f"adj{bb}_{t}") for t in range(NT)]
             for bb in range(BPC)]
    for bb, t in [(0, 0), (1, 0), (0, 1), (1, 1), (2, 0), (3, 0), (2, 1), (3, 1)]:
        nc.sync.dma_start(adj_t[bb][t][:], adjT8[bb, t])

    # ---- emission helpers --------------------------------------------
    hs = [None] * BPC    # current hT tile per graph
    msgs = [None] * BPC  # current msg tile per graph

    def emit_bias_bcast(i):
        # bias row -> all 128 partitions via a rank-1 fp16 matmul; the row
        # arrives pre-scaled by 2^14 so fp16 never subnormalizes, and the
        # ACT copy undoes the (exact power-of-2) factor.
        pb = psM.tile([128, 4 * HID], FP32, tag="psM", name=f"pb{i}")
        nc.tensor.matmul(pb[:], ones_h[:], blr_t[i][:], start=True, stop=True)
        nc.scalar.mul(bl_t[i][:], pb[:], 2.0 ** -14)

    def emit_embed(bb):
        h0 = h_pool.tile([HID, N], FP16, tag="h", name=f"h0_{bb}")
        for t in range(NT):
            ps = psA.tile([HID, 512], FP32, tag="psA")
            nc.tensor.matmul(ps[:], we_t[:], xts[bb][:, t * 512:(t + 1) * 512],
                             start=True, stop=True)
            nc.scalar.copy(h0[:, t * 512:(t + 1) * 512], ps[:])
        hs[bb] = h0

    def make_msg_tile(bb, i):
        msgs[bb] = msg_pool.tile([128, NC8, HID], FP8, tag="msg",
                                 name=f"msg{bb}_{i}")

    def emit_msg_half(bb, i, half):
        # msg[n, k] = h @ Wl[i]; 4 node-chunks of 128 share one PSUM bank,
        # one DVE op adds the (4x-tiled) bias and casts to fp8.
        h, msg_t = hs[bb], msgs[bb]
        pm = psM.tile([128, 4 * HID], FP32, tag="psM")
        for j in range(4):
            c = 4 * half + j
            nc.tensor.matmul(pm[:, j * HID:(j + 1) * HID],
                             h[:, c * 128:(c + 1) * 128], wl_t[i][:],
                             start=True, stop=True)
        nc.vector.tensor_add(msg_t[:, 4 * half:4 * half + 4, :], pm[:], bl_t[i][:])

    def emit_agg(bb, i, t, h_new, msg_t):
        # aggT[k, n-tile] = msg.T @ adjT via 4 fp8 DoubleRow matmuls
        # (each contracts 2 node-chunks = 256 sources); relu + rescale.
        adj = adj_t[bb][t]
        ps = psA.tile([HID, 512], FP32, tag="psA")
        for c in range(4):
            nc.tensor.matmul(ps[:], msg_t[:, 2 * c:2 * c + 2, :],
                             adj[:, 2 * c:2 * c + 2, :],
                             start=(c == 0), stop=(c == 3), perf_mode=DR)
        nc.scalar.activation(h_new[:, t * 512:(t + 1) * 512], ps[:], RELU,
                             scale=S[i] / S[i + 1])

    o_ts = [None] * BPC

    def emit_proj_half(bb, t):
        if o_ts[bb] is None:
            o_ts[bb] = o_pool.tile([OUT, N], FP16, tag="o", name=f"o{bb}")
        o_t = o_ts[bb]
        po = psO.tile([OUT, 512], FP32, tag="psO")
        nc.tensor.matmul(po[:], wp_t[:], hs[bb][:, t * 512:(t + 1) * 512],
                         start=True, stop=True)
        nc.vector.tensor_scalar_add(o_t[:, t * 512:(t + 1) * 512], po[:], 0.0)
        nc.gpsimd.dma_start(out[bb, :, t * 512:(t + 1) * 512],
                            o_t[:, t * 512:(t + 1) * 512])

    # ---- prologue: embed + msg-layer-0 for pair (0,1) ----------------
    # (pair (2,3)'s embed/msg0 and the later bias broadcasts interleave
    # into pair (0,1)'s rounds as `extra` units.)
    emit_bias_bcast(0)
    emit_embed(0)
    emit_embed(1)
    for bb in (0, 1):
        make_msg_tile(bb, 0)
        emit_msg_half(bb, 0, 0)
        emit_msg_half(bb, 0, 1)

    # ---- paired rounds ------------------------------------------------
    # Round (A, B, layer i): A/B agg groups alternate so each graph's
    # relu -> msg matmul -> DVE-quantize chain hides under the other
    # graph's DoubleRow aggregations. On the last layer the msg slots
    # carry the projection instead. `extra` units (next pair's
    # embed/msg0, bias broadcasts) fill the remaining slack.
    def round_(A, BQ, i, extra):
        ex = list(extra)

        def drain(k):
            for _ in range(k):
                if ex:
                    ex.pop(0)()

        hA = h_pool.tile([HID, N], FP16, tag="h", name=f"h{i + 1}_{A}")
        hB = h_pool.tile([HID, N], FP16, tag="h", name=f"h{i + 1}_{BQ}")
        msgA, msgB = msgs[A], msgs[BQ]
        last = i == NUM_LAYERS - 1
        emit_agg(A, i, 0, hA, msgA)
        drain(1)
        emit_agg(BQ, i, 0, hB, msgB)
        hs[A] = hA
        if not last:
            make_msg_tile(A, i + 1)
            emit_msg_half(A, i + 1, 0)  # needs relu(A, t0): done during B.t0
        else:
            emit_proj_half(A, 0)
        emit_agg(A, i, 1, hA, msgA)
        drain(1)
        hs[BQ] = hB
        if not last:
            make_msg_tile(BQ, i + 1)
            emit_msg_half(BQ, i + 1, 0)
        else:
            emit_proj_half(BQ, 0)
        emit_agg(BQ, i, 1, hB, msgB)
        if not last:
            emit_msg_half(A, i + 1, 1)
            drain(1)
            emit_msg_half(BQ, i + 1, 1)
        else:
            emit_proj_half(A, 1)
            emit_proj_half(BQ, 1)
        drain(len(ex))

    def units_embed_msg0(bb):
        def u1():
            emit_embed(bb)

        def u2():
            make_msg_tile(bb, 0)
            emit_msg_half(bb, 0, 0)
            emit_msg_half(bb, 0, 1)

        return [u1, u2]

    round_(0, 1, 0, [lambda: emit_bias_bcast(1)] + units_embed_msg0(2))
    round_(0, 1, 1, [lambda: emit_bias_bcast(2)] + units_embed_msg0(3))
    round_(0, 1, 2, [])
    round_(2, 3, 0, [])
    round_(2, 3, 1, [])
    round_(2, 3, 2, [])


def build_nc():
    # Bacc (not raw Bass): its compile() runs generate_event_semaphores,
    # which splits multi-sem waits down to the 1-wait-per-instruction
    # hardware limit walrus enforces.
    nc = bacc.Bacc("TRN2", debug=False, num_devices=N_CORES, num_swdge_queues=2)
    adjT8 = nc.dram_tensor("adjT8", [BPC, NT, 128, NC8, 512], FP8,
                           kind="ExternalInput").ap()
    xT = nc.dram_tensor("xT", [BPC, IN_DIM, N], FP16, kind="ExternalInput").ap()
    w_embed = nc.dram_tensor("w_embed", [IN_DIM, HID], FP16,
                             kind="ExternalInput").ap()
    wl = nc.dram_tensor("wl", [NUM_LAYERS, HID, HID], FP16,
                        kind="ExternalInput").ap()
    blrT = nc.dram_tensor("blrT", [NUM_LAYERS, 1, 4 * HID], FP16,
                          kind="ExternalInput").ap()
    w_proj = nc.dram_tensor("w_proj", [HID, OUT], FP16, kind="ExternalInput").ap()
    out = nc.dram_tensor("out", [BPC, OUT, N], FP16, kind="ExternalOutput").ap()

    with tile.TileContext(nc) as tc, ExitStack() as ctx:
        _kernel_body(ctx, tc, out, adjT8, xT, w_embed, wl, blrT, w_proj)
    nc.compile()
    return nc


def make_in_maps(node_features, adjacency_matrix, node_mask, W_embed, Wl, bl,
                 W_proj, b_proj):
    x = np.asarray(node_features, dtype=np.float32)
    adj = np.asarray(adjacency_matrix, dtype=np.float32)
    bl4 = np.tile(
        (np.asarray(bl, np.float64) * 2.0 ** 14
         / np.array(S[:NUM_LAYERS])[:, None]), (1, 4)
    ).astype(np.float16).reshape(NUM_LAYERS, 1, 4 * HID)
    shared = {
        "w_embed": np.asarray(W_embed, dtype=np.float16),
        "wl": np.asarray(Wl, dtype=np.float16),
        "blrT": bl4,
        "w_proj": np.asarray(W_proj, dtype=np.float16),
    }
    in_maps = []
    for c in range(N_CORES):
        sl = slice(c * BPC, (c + 1) * BPC)
        # adjT8[bb, t, p, c, j] = adj[bb, t*512+j, c*128+p]
        a = adj[sl].reshape(BPC, NT, 512, NC8, 128).transpose(0, 1, 4, 3, 2)
        in_maps.append({
            "adjT8": np.ascontiguousarray(a).astype(ml_dtypes.float8_e4m3fn),
            "xT": np.ascontiguousarray(x[sl].transpose(0, 2, 1)).astype(np.float16),
            **shared,
        })
    return in_maps


_NC_CACHE = None


def get_nc():
    global _NC_CACHE
    if _NC_CACHE is None:
        _NC_CACHE = build_nc()
    return _NC_CACHE


def kernel(**inputs):
    nc = get_nc()
    in_maps = make_in_maps(**inputs)
    res = run_bass_kernel_spmd(nc, in_maps, list(range(N_CORES)))
    outs = [np.asarray(res.results[c]["out"]) for c in range(N_CORES)]
    oT = np.concatenate(outs, axis=0).astype(np.float32)  # [B, OUT, N]
    o = oT.transpose(0, 2, 1) * np.float32(S[NUM_LAYERS])
    o = o + np.asarray(inputs["b_proj"], np.float32)
    o = o * np.asarray(inputs["node_mask"], np.float32)
    return o


if __name__ == "__main__":
    rng = np.random.default_rng(0)
    ins = {
        "node_features": rng.standard_normal((B, N, IN_DIM), dtype=np.float32),
        "adjacency_matrix": rng.random((B, N, N), dtype=np.float32),
        "node_mask": np.ones((B, N, 1), np.float32),
        "W_embed": rng.standard_normal((IN_DIM, HID), dtype=np.float32) * 0.1,
        "Wl": rng.standard_normal((NUM_LAYERS, HID, HID), dtype=np.float32) * 0.08,
        "bl": rng.standard_normal((NUM_LAYERS, HID), dtype=np.float32) * 0.08,
        "W_proj": rng.standard_normal((HID, 2 * 32), dtype=np.float32) * 0.08,
        "b_proj": rng.standard_normal((2 * 32,), dtype=np.float32) * 0.08,
    }
    out = kernel(**ins)
    print("out", out.shape, out.dtype, float(np.abs(out).mean()))


# revision 8
# speedup vs baseline: 1.1482x; 1.1482x over previous
"""Bass/Trainium2 kernel for nn_EuclideanGraphEncoder (GCN message passing).

Strategy: data-parallel over the batch (4 graphs per core, 8 cores),
weights replicated, no collectives. The dominant aggregation matmul
(adj @ msg) runs in fp8-e4m3 DoubleRow mode (2 k-tiles per instruction,
2x fp8 throughput): the adjacency is downcast+transposed to fp8 on the
host, and the per-layer messages are quantized to fp8 on the DVE during
the PSUM->SBUF bias-add. Node features h stay fp16 (their quantization
error would hit the output un-averaged), so the final rel-err keeps the
adjacency/message fp8 noise only, which the 1024-wide sum-aggregation
averages down to ~1.5e-2.

Device-side layout: h is kept transposed [hid=128 partitions, n=1024]
fp16. Per layer:
  msg[n,k]  = h @ Wl          (8x K=128 matmuls; DVE adds bias, casts fp8)
  aggT[k,n] = msg.T @ adjT    (2 n-tiles x 4 DoubleRow fp8 matmuls)
  hT        = relu(aggT)      (ACT, PSUM->SBUF fp16, power-of-2 rescale)
Projection keeps the transposed layout (W_proj stationary, hT moving,
N=512) and returns outT [64, n] fp16 scaled by 1/S3; the host transposes,
rescales, adds b_proj and applies the node mask.

Scheduling notes (from NTFF profiles): DMA completion semaphores lag
their bytes by 2-4us each during the ring ramp and serialize per queue,
so every small tensor the prologue needs ships as ONE packed transfer
(one semaphore); full-array dependency-free matmuls spin the PE through
the HAM clock ramp (1.2 -> 2.4 GHz, ~3.4us of sustained full-array
activity -- rank-1 warm-ups do NOT trip the monitor) while the rings
spin up; the per-layer biases ride the pack as [1, 512] rows and are
partition-broadcast by a rank-1 fp16 matmul; graphs run in
software-pipelined pairs so each graph's relu -> msg -> fp8-quantize
chain hides under the other graph's DoubleRow aggregations; out stores
trigger from SWDGE so the ACT engine never stalls on descriptor
generation.
"""

import sys
from contextlib import ExitStack

import ml_dtypes
import numpy as np

try:
    import concourse.bass as bass
except ImportError:  # fall back to the repo checkout
    sys.path.insert(0, "/opt/trn_rl_repo")
    import concourse.bass as bass

import concourse.tile as tile
from concourse import bacc, mybir
from concourse.bass_utils import run_bass_kernel_spmd

B, N, IN_DIM, HID, OUT = 32, 1024, 64, 128, 64
NUM_LAYERS = 3
N_CORES = 8
BPC = B // N_CORES  # graphs per core
NT = N // 512  # aggregation free-dim tiles
NC8 = N // 128  # node chunks of 128

FP8 = mybir.dt.float8e4
FP16 = mybir.dt.float16
FP32 = mybir.dt.float32
RELU = mybir.ActivationFunctionType.Relu
DR = mybir.MatmulPerfMode.DoubleRow

# Per-layer power-of-2 scales: SBUF h/msg tiles hold h_true / S[i] so fp16
# never overflows (true agg magnitudes reach ~5e6). Scale hops are exact
# (powers of two) and ride the relu ACT scale; the host pre-scales the
# biases and post-scales the output.
S = [1.0, 64.0, 16384.0, 4194304.0]
BLZ = 2.0 ** 14  # bias rows pre-scaled so fp16 never subnormalizes

# pack layout (fp16 columns): every small tensor the prologue depends on,
# delivered by a single DMA so one completion semaphore gates it all.
PK_WE = 0                  # [0:64,   0:128]   W_embed
PK_WL = 128                # [0:128,  128+128i] Wl[i]
PK_WP = 512                # [0:128,  512:576] W_proj
PK_BL = 576                # [0:1,    576+512i] bias rows (x4 tiled, x BLZ)
PK_X0 = PK_BL + 3 * 512    # [0:64,   2112:3136] x of graph 0
PACKW = PK_X0 + N

WARM_MMS = 8  # PE clock warm-up matmuls covering the DMA-ring spin-up


def _kernel_body(ctx, tc, out, adjT8, packD, xT):
    nc = tc.nc

    consts = ctx.enter_context(tc.tile_pool(name="consts", bufs=1))
    adj_pool = ctx.enter_context(tc.tile_pool(name="adj", bufs=BPC * NT))
    xt_pool = ctx.enter_context(tc.tile_pool(name="xt", bufs=BPC - 1))
    h_pool = ctx.enter_context(tc.tile_pool(name="h", bufs=8))
    msg_pool = ctx.enter_context(tc.tile_pool(name="msg", bufs=6))
    o_pool = ctx.enter_context(tc.tile_pool(name="o", bufs=BPC))
    psA = ctx.enter_context(tc.tile_pool(name="psA", bufs=3, space="PSUM"))
    psM = ctx.enter_context(tc.tile_pool(name="psM", bufs=3, space="PSUM"))
    psO = ctx.enter_context(tc.tile_pool(name="psO", bufs=2, space="PSUM"))

    # ---- PE clock pre-warm -------------------------------------------
    # Dependency-free FULL-ARRAY matmuls from t=0: the HAM un-throttles
    # (1.2 -> 2.4 GHz) after ~3.4us of sustained PE activity, so the real
    # work (gated on the first DMA deliveries at ~9us) starts at full
    # clock. K=1 warm-ups don't register on the activity monitor.
    warm_w = consts.tile([128, HID], FP16, tag="warm_w")
    warm_m = consts.tile([128, 512], FP16, tag="warm_m")
    nc.vector.memset(warm_w[:], 0.0)
    nc.vector.memset(warm_m[:], 0.0)
    ones_h = consts.tile([1, HID], FP16, tag="ones_h")
    nc.vector.memset(ones_h[:], 1.0)
    for w in range(WARM_MMS):
        psw = psA.tile([HID, 512], FP32, tag="psA", name="psw")
        nc.tensor.matmul(psw[:], warm_w[:], warm_m[:], start=True, stop=True)

    # ---- loads --------------------------------------------------------
    # Sync HWDGE ring: the pack, then the adj flood. The second HWDGE
    # ring (scalar) carries x1 in parallel; x2/x3 ride SWDGE (gpsimd),
    # which also triggers the out stores.
    pack_t = consts.tile([128, PACKW], FP16, tag="pack")
    nc.sync.dma_start(pack_t[:], packD[:, :])

    xts = [None] + [xt_pool.tile([IN_DIM, N], FP16, tag="xt", name=f"xt{bb}")
                    for bb in (1, 2, 3)]
    nc.scalar.dma_start(xts[1][:], xT[1])
    for bb in (2, 3):
        nc.gpsimd.dma_start(xts[bb][:], xT[bb])

    def x_ap(bb):
        return pack_t[0:64, PK_X0:PK_X0 + N] if bb == 0 else xts[bb][:]

    we_ap = pack_t[0:64, PK_WE:PK_WE + HID]
    wl_ap = [pack_t[:, PK_WL + HID * i:PK_WL + HID * (i + 1)]
             for i in range(NUM_LAYERS)]
    wp_ap = pack_t[:, PK_WP:PK_WP + OUT]
    blr_ap = [pack_t[0:1, PK_BL + 512 * i:PK_BL + 512 * (i + 1)]
              for i in range(NUM_LAYERS)]
    bl_t = [consts.tile([128, 4 * HID], FP32, tag=f"bl{i}", name=f"bl{i}")
            for i in range(NUM_LAYERS)]

    # adj: one 512KB DMA per (graph, n-half); 4KB contiguous per partition.
    adj_t = [[adj_pool.tile([128, NC8, 512], FP8, tag="adj",
                            name=f"adj{bb}_{t}") for t in range(NT)]
             for bb in range(BPC)]
    for bb, t in [(0, 0), (1, 0), (0, 1), (1, 1), (2, 0), (3, 0), (2, 1), (3, 1)]:
        nc.sync.dma_start(adj_t[bb][t][:], adjT8[bb, t])

    # ---- emission helpers --------------------------------------------
    hs = [None] * BPC    # current hT tile per graph
    msgs = [None] * BPC  # current msg tile per graph

    def emit_bias_bcast(i):
        # bias row -> all 128 partitions via a rank-1 fp16 matmul; the row
        # is pre-scaled by 2^14 so fp16 never subnormalizes, and the ACT
        # copy undoes the (exact power-of-2) factor.
        pb = psM.tile([128, 4 * HID], FP32, tag="psM", name=f"pb{i}")
        nc.tensor.matmul(pb[:], ones_h[:], blr_ap[i], start=True, stop=True)
        nc.scalar.mul(bl_t[i][:], pb[:], 1.0 / BLZ)

    def emit_embed(bb):
        h0 = h_pool.tile([HID, N], FP16, tag="h", name=f"h0_{bb}")
        x = x_ap(bb)
        for t in range(NT):
            ps = psA.tile([HID, 512], FP32, tag="psA")
            nc.tensor.matmul(ps[:], we_ap, x[:, t * 512:(t + 1) * 512],
                             start=True, stop=True)
            nc.scalar.copy(h0[:, t * 512:(t + 1) * 512], ps[:])
        hs[bb] = h0

    def make_msg_tile(bb, i):
        msgs[bb] = msg_pool.tile([128, NC8, HID], FP8, tag="msg",
                                 name=f"msg{bb}_{i}")

    def emit_msg_half(bb, i, half):
        # msg[n, k] = h @ Wl[i]; 4 node-chunks of 128 share one PSUM bank,
        # one DVE op adds the (4x-tiled) bias and casts to fp8.
        h, msg_t = hs[bb], msgs[bb]
        pm = psM.tile([128, 4 * HID], FP32, tag="psM")
        for j in range(4):
            c = 4 * half + j
            nc.tensor.matmul(pm[:, j * HID:(j + 1) * HID],
                             h[:, c * 128:(c + 1) * 128], wl_ap[i],
                             start=True, stop=True)
        nc.vector.tensor_add(msg_t[:, 4 * half:4 * half + 4, :], pm[:], bl_t[i][:])

    def emit_agg(bb, i, t, h_new, msg_t):
        # aggT[k, n-tile] = msg.T @ adjT via 4 fp8 DoubleRow matmuls
        # (each contracts 2 node-chunks = 256 sources); relu + rescale.
        adj = adj_t[bb][t]
        ps = psA.tile([HID, 512], FP32, tag="psA")
        for c in range(4):
            nc.tensor.matmul(ps[:], msg_t[:, 2 * c:2 * c + 2, :],
                             adj[:, 2 * c:2 * c + 2, :],
                             start=(c == 0), stop=(c == 3), perf_mode=DR)
        nc.scalar.activation(h_new[:, t * 512:(t + 1) * 512], ps[:], RELU,
                             scale=S[i] / S[i + 1])

    o_ts = [None] * BPC

    def emit_proj_half(bb, t, split=False):
        if o_ts[bb] is None:
            o_ts[bb] = o_pool.tile([OUT, N], FP16, tag="o", name=f"o{bb}")
        o_t = o_ts[bb]
        po = psO.tile([OUT, 512], FP32, tag="psO")
        nc.tensor.matmul(po[:], wp_ap, hs[bb][:, t * 512:(t + 1) * 512],
                         start=True, stop=True)
        lo = t * 512
        if split:
            # tail-exposed halves: ACT and DVE copy a quarter each so the
            # store can trigger ~2x sooner.
            nc.scalar.copy(o_t[:, lo:lo + 256], po[:, 0:256])
            nc.vector.tensor_scalar_add(o_t[:, lo + 256:lo + 512],
                                        po[:, 256:512], 0.0)
        else:
            nc.vector.tensor_scalar_add(o_t[:, lo:lo + 512], po[:], 0.0)
        nc.gpsimd.dma_start(out[bb, :, lo:lo + 512], o_t[:, lo:lo + 512])

    # ---- prologue: embed + msg-layer-0 for pair (0,1) ----------------
    # (pair (2,3)'s embed/msg0 and the later bias broadcasts interleave
    # into pair (0,1)'s rounds as `extra` units.)
    emit_bias_bcast(0)
    emit_embed(0)
    emit_embed(1)
    for bb in (0, 1):
        make_msg_tile(bb, 0)
        emit_msg_half(bb, 0, 0)
        emit_msg_half(bb, 0, 1)

    # ---- paired rounds ------------------------------------------------
    # Round (A, B, layer i): A/B agg groups alternate so each graph's
    # relu -> msg matmul -> DVE-quantize chain hides under the other
    # graph's DoubleRow aggregations. On the last layer the msg slots
    # carry the projection instead. `extra` units (next pair's
    # embed/msg0, bias broadcasts) fill the remaining slack.
    def round_(A, BQ, i, extra, split_tail=False):
        ex = list(extra)

        def drain(k):
            for _ in range(k):
                if ex:
                    ex.pop(0)()

        hA = h_pool.tile([HID, N], FP16, tag="h", name=f"h{i + 1}_{A}")
        hB = h_pool.tile([HID, N], FP16, tag="h", name=f"h{i + 1}_{BQ}")
        msgA, msgB = msgs[A], msgs[BQ]
        last = i == NUM_LAYERS - 1
        emit_agg(A, i, 0, hA, msgA)
        drain(1)
        emit_agg(BQ, i, 0, hB, msgB)
        hs[A] = hA
        if not last:
            make_msg_tile(A, i + 1)
            emit_msg_half(A, i + 1, 0)  # needs relu(A, t0): done during B.t0
        else:
            emit_proj_half(A, 0)
        emit_agg(A, i, 1, hA, msgA)
        drain(1)
        hs[BQ] = hB
        if not last:
            make_msg_tile(BQ, i + 1)
            emit_msg_half(BQ, i + 1, 0)
        else:
            emit_proj_half(BQ, 0)
        emit_agg(BQ, i, 1, hB, msgB)
        if not last:
            emit_msg_half(A, i + 1, 1)
            drain(1)
            emit_msg_half(BQ, i + 1, 1)
        else:
            emit_proj_half(A, 1, split=split_tail)
            emit_proj_half(BQ, 1, split=split_tail)
        drain(len(ex))

    def units_embed_msg0(bb):
        def u1():
            emit_embed(bb)

        def u2():
            make_msg_tile(bb, 0)
            emit_msg_half(bb, 0, 0)
            emit_msg_half(bb, 0, 1)

        return [u1, u2]

    round_(0, 1, 0, [lambda: emit_bias_bcast(1)] + units_embed_msg0(2))
    round_(0, 1, 1, [lambda: emit_bias_bcast(2)] + units_embed_msg0(3))
    round_(0, 1, 2, [])
    round_(2, 3, 0, [])
    round_(2, 3, 1, [])
    round_(2, 3, 2, [], split_tail=True)


def build_nc():
    # Bacc (not raw Bass): its compile() runs generate_event_semaphores,
    # which splits multi-sem waits down to the 1-wait-per-instruction
    # hardware limit walrus enforces.
    nc = bacc.Bacc("TRN2", debug=False, num_devices=N_CORES, num_swdge_queues=2)
    adjT8 = nc.dram_tensor("adjT8", [BPC, NT, 128, NC8, 512], FP8,
                           kind="ExternalInput").ap()
    packD = nc.dram_tensor("pack", [128, PACKW], FP16, kind="ExternalInput").ap()
    xT = nc.dram_tensor("xT", [BPC, IN_DIM, N], FP16, kind="ExternalInput").ap()
    out = nc.dram_tensor("out", [BPC, OUT, N], FP16, kind="ExternalOutput").ap()

    with tile.TileContext(nc) as tc, ExitStack() as ctx:
        _kernel_body(ctx, tc, out, adjT8, packD, xT)
    nc.compile()
    return nc


def make_in_maps(node_features, adjacency_matrix, node_mask, W_embed, Wl, bl,
                 W_proj, b_proj):
    x = np.asarray(node_features, dtype=np.float32)
    adj = np.asarray(adjacency_matrix, dtype=np.float32)
    bl4 = np.tile(
        (np.asarray(bl, np.float64) * BLZ / np.array(S[:NUM_LAYERS])[:, None]),
        (1, 4))
    in_maps = []
    for c in range(N_CORES):
        sl = slice(c * BPC, (c + 1) * BPC)
        xTc = np.ascontiguousarray(x[sl].transpose(0, 2, 1)).astype(np.float16)
        pack = np.zeros((128, PACKW), np.float16)
        pack[0:64, PK_WE:PK_WE + HID] = np.asarray(W_embed, np.float16)
        for i in range(NUM_LAYERS):
            pack[:, PK_WL + HID * i:PK_WL + HID * (i + 1)] = np.asarray(
                Wl[i], np.float16)
            pack[0, PK_BL + 512 * i:PK_BL + 512 * (i + 1)] = bl4[i].astype(
                np.float16)
        pack[:, PK_WP:PK_WP + OUT] = np.asarray(W_proj, np.float16)
        pack[0:64, PK_X0:PK_X0 + N] = xTc[0]
        # adjT8[bb, t, p, c, j] = adj[bb, t*512+j, c*128+p]
        a = adj[sl].reshape(BPC, NT, 512, NC8, 128).transpose(0, 1, 4, 3, 2)
        in_maps.append({
            "adjT8": np.ascontiguousarray(a).astype(ml_dtypes.float8_e4m3fn),
            "pack": pack,
            "xT": xTc,
        })
    return in_maps


_NC_CACHE = None


def get_nc():
    global _NC_CACHE
    if _NC_CACHE is None:
        _NC_CACHE = build_nc()
    return _NC_CACHE


def kernel(**inputs):
    nc = get_nc()
    in_maps = make_in_maps(**inputs)
    res = run_bass_kernel_spmd(nc, in_maps, list(range(N_CORES)))
    outs = [np.asarray(res.results[c]["out"]) for c in range(N_CORES)]
    oT = np.concatenate(outs, axis=0).astype(np.float32)  # [B, OUT, N]
    o = oT.transpose(0, 2, 1) * np.float32(S[NUM_LAYERS])
    o = o + np.asarray(inputs["b_proj"], np.float32)
    o = o * np.asarray(inputs["node_mask"], np.float32)
    return o


if __name__ == "__main__":
    rng = np.random.default_rng(0)
    ins = {
        "node_features": rng.standard_normal((B, N, IN_DIM), dtype=np.float32),
        "adjacency_matrix": rng.random((B, N, N), dtype=np.float32),
        "node_mask": np.ones((B, N, 1), np.float32),
        "W_embed": rng.standard_normal((IN_DIM, HID), dtype=np.float32) * 0.1,
        "Wl": rng.standard_normal((NUM_LAYERS, HID, HID), dtype=np.float32) * 0.08,
        "bl": rng.standard_normal((NUM_LAYERS, HID), dtype=np.float32) * 0.08,
        "W_proj": rng.standard_normal((HID, 2 * 32), dtype=np.float32) * 0.08,
        "b_proj": rng.standard_normal((2 * 32,), dtype=np.float32) * 0.08,
    }
    out = kernel(**ins)
    print("out", out.shape, out.dtype, float(np.abs(out).mean()))


# revision 13
# speedup vs baseline: 1.1646x; 1.0143x over previous
"""Bass/Trainium2 kernel for nn_EuclideanGraphEncoder (GCN message passing).

Strategy: data-parallel over the batch (4 graphs per core, 8 cores),
weights replicated, no collectives. The dominant aggregation matmul
(adj @ msg) runs in fp8-e4m3 DoubleRow mode (2 k-tiles per instruction,
2x fp8 throughput): the adjacency is downcast+transposed to fp8 on the
host, and the per-layer messages are quantized to fp8 on the DVE during
the PSUM->SBUF bias-add. Node features h stay fp16 (their quantization
error would hit the output un-averaged), so the final rel-err keeps the
adjacency/message fp8 noise only, which the 1024-wide sum-aggregation
averages down to ~1.5e-2.

Device-side layout: h is kept transposed [hid=128 partitions, n=1024]
fp16. Per layer:
  msg[n,k]  = h @ Wl          (8x K=128 matmuls; DVE adds bias, casts fp8)
  aggT[k,n] = msg.T @ adjT    (2 n-tiles x 4 DoubleRow fp8 matmuls)
  hT        = relu(aggT)      (ACT, PSUM->SBUF fp16, power-of-2 rescale)
Projection keeps the transposed layout (W_proj stationary, hT moving,
N=512) and returns outT [64, n] fp16 scaled by 1/S3; the host transposes,
rescales, adds b_proj and applies the node mask.

Scheduling notes (from NTFF profiles): DMA completion semaphores lag
their bytes by 2-4us each during the ring ramp and serialize per queue,
so every small tensor the prologue needs ships as ONE packed transfer
(one semaphore); full-array dependency-free matmuls spin the PE through
the HAM clock ramp (1.2 -> 2.4 GHz, ~3.4us of sustained full-array
activity -- rank-1 warm-ups do NOT trip the monitor) while the rings
spin up; the per-layer biases ride the pack as [1, 512] rows and are
partition-broadcast by a rank-1 fp16 matmul; graphs run in
software-pipelined pairs so each graph's relu -> msg -> fp8-quantize
chain hides under the other graph's DoubleRow aggregations; out stores
trigger from SWDGE so the ACT engine never stalls on descriptor
generation.
"""

import sys
from contextlib import ExitStack

import ml_dtypes
import numpy as np

try:
    import concourse.bass as bass
except ImportError:  # fall back to the repo checkout
    sys.path.insert(0, "/opt/trn_rl_repo")
    import concourse.bass as bass

import concourse.tile as tile
from concourse import bacc, mybir
from concourse.bass_utils import run_bass_kernel_spmd

B, N, IN_DIM, HID, OUT = 32, 1024, 64, 128, 64
NUM_LAYERS = 3
N_CORES = 8
BPC = B // N_CORES  # graphs per core
NT = N // 512  # aggregation free-dim tiles
NC8 = N // 128  # node chunks of 128

FP8 = mybir.dt.float8e4
FP16 = mybir.dt.float16
FP32 = mybir.dt.float32
RELU = mybir.ActivationFunctionType.Relu
DR = mybir.MatmulPerfMode.DoubleRow

# Per-layer power-of-2 scales: SBUF h/msg tiles hold h_true / S[i] so fp16
# never overflows (true agg magnitudes reach ~5e6). Scale hops are exact
# (powers of two) and ride the relu ACT scale; the host pre-scales the
# biases and post-scales the output.
S = [1.0, 64.0, 16384.0, 4194304.0]
BLZ = 2.0 ** 14  # bias rows pre-scaled so fp16 never subnormalizes

# pack layouts (fp16 columns): the small tensors the prologue depends on
# ship as TWO packed transfers -- one per HWDGE ring -- so two completion
# semaphores (which lag their bytes by 2-4us during the ring ramp and
# serialize per queue) gate everything in parallel.
# pack_a (sync ring, [64, *]): W_embed + x of graph 0.
PKA_WE = 0
PKA_X0 = HID
PACKA_W = PKA_X0 + N
# pack_b (scalar ring, [128, *]): Wl, W_proj, bias rows, x of graph 1.
PKB_WL = 0                  # [0:128, 128i:128(i+1)]
PKB_WP = 3 * HID            # [0:128, 384:448]
PKB_BL = PKB_WP + OUT       # [0:1,   448+512i]  (x4 tiled, x BLZ)
PKB_X1 = PKB_BL + 3 * 512   # [0:64,  1984:3008]
PACKB_W = PKB_X1 + N

WARM_MMS = 10  # PE clock warm-up matmuls covering the DMA-ring spin-up


def _kernel_body(ctx, tc, out, adjT8, packA, packB, xT):
    nc = tc.nc

    consts = ctx.enter_context(tc.tile_pool(name="consts", bufs=1))
    adj_pool = ctx.enter_context(tc.tile_pool(name="adj", bufs=BPC * NT))
    xt_pool = ctx.enter_context(tc.tile_pool(name="xt", bufs=BPC - 1))
    h_pool = ctx.enter_context(tc.tile_pool(name="h", bufs=8))
    msg_pool = ctx.enter_context(tc.tile_pool(name="msg", bufs=6))
    o_pool = ctx.enter_context(tc.tile_pool(name="o", bufs=BPC))
    psA = ctx.enter_context(tc.tile_pool(name="psA", bufs=3, space="PSUM"))
    psM = ctx.enter_context(tc.tile_pool(name="psM", bufs=3, space="PSUM"))
    psO = ctx.enter_context(tc.tile_pool(name="psO", bufs=2, space="PSUM"))

    # ---- PE clock pre-warm -------------------------------------------
    # Dependency-free FULL-ARRAY matmuls from t=0: the HAM un-throttles
    # (1.2 -> 2.4 GHz) after ~3.4us of sustained PE activity, so the real
    # work (gated on the first DMA deliveries at ~9us) starts at full
    # clock. K=1 warm-ups don't register on the activity monitor.
    warm_w = consts.tile([128, HID], FP16, tag="warm_w")
    warm_m = consts.tile([128, 512], FP16, tag="warm_m")
    nc.vector.memset(warm_w[:], 0.7071)
    # varied column bands: the moving operand must toggle the PE datapath
    # cycle-to-cycle or the activity monitor won't count the work.
    for j, v in enumerate([1.0, -777.0, 0.013, 3e4, -2.0e-4, 255.0, -0.5, 41.0]):
        nc.vector.memset(warm_m[:, j * 64:(j + 1) * 64], v)
    ones_h = consts.tile([1, HID], FP16, tag="ones_h")
    nc.vector.memset(ones_h[:], 1.0)
    for w in range(WARM_MMS):
        psw = psA.tile([HID, 512], FP32, tag="psA", name="psw")
        nc.tensor.matmul(psw[:], warm_w[:], warm_m[:], start=True, stop=True)

    # ---- loads --------------------------------------------------------
    # Sync HWDGE ring: the pack, then the adj flood. The second HWDGE
    # ring (scalar) carries x1 in parallel; x2/x3 ride SWDGE (gpsimd),
    # which also triggers the out stores.
    pka_t = consts.tile([64, PACKA_W], FP16, tag="pka")
    nc.sync.dma_start(pka_t[:], packA[:, :])
    pkb_t = consts.tile([128, PACKB_W], FP16, tag="pkb")
    nc.scalar.dma_start(pkb_t[:], packB[:, :])

    xts = [None, None] + [xt_pool.tile([IN_DIM, N], FP16, tag="xt",
                                       name=f"xt{bb}") for bb in (2, 3)]
    for bb in (2, 3):
        nc.gpsimd.dma_start(xts[bb][:], xT[bb])

    def x_ap(bb):
        if bb == 0:
            return pka_t[0:64, PKA_X0:PKA_X0 + N]
        if bb == 1:
            return pkb_t[0:64, PKB_X1:PKB_X1 + N]
        return xts[bb][:]

    we_ap = pka_t[0:64, PKA_WE:PKA_WE + HID]
    wl_ap = [pkb_t[:, PKB_WL + HID * i:PKB_WL + HID * (i + 1)]
             for i in range(NUM_LAYERS)]
    wp_ap = pkb_t[:, PKB_WP:PKB_WP + OUT]
    blr_ap = [pkb_t[0:1, PKB_BL + 512 * i:PKB_BL + 512 * (i + 1)]
              for i in range(NUM_LAYERS)]
    bl_t = [consts.tile([128, 4 * HID], FP32, tag=f"bl{i}", name=f"bl{i}")
            for i in range(NUM_LAYERS)]

    # adj: one 512KB DMA per (graph, n-half); 4KB contiguous per partition.
    adj_t = [[adj_pool.tile([128, NC8, 512], FP8, tag="adj",
                            name=f"adj{bb}_{t}") for t in range(NT)]
             for bb in range(BPC)]
    for bb, t in [(0, 0), (1, 0), (0, 1), (1, 1), (2, 0), (3, 0), (2, 1), (3, 1)]:
        nc.sync.dma_start(adj_t[bb][t][:], adjT8[bb, t])

    # ---- emission helpers --------------------------------------------
    hs = [None] * BPC    # current hT tile per graph
    msgs = [None] * BPC  # current msg tile per graph

    def emit_bias_bcast(i):
        # bias row -> all 128 partitions via a rank-1 fp16 matmul; the row
        # is pre-scaled by 2^14 so fp16 never subnormalizes, and the ACT
        # copy undoes the (exact power-of-2) factor.
        pb = psM.tile([128, 4 * HID], FP32, tag="psM", name=f"pb{i}")
        nc.tensor.matmul(pb[:], ones_h[:], blr_ap[i], start=True, stop=True)
        nc.scalar.mul(bl_t[i][:], pb[:], 1.0 / BLZ)

    def emit_embed(bb):
        h0 = h_pool.tile([HID, N], FP16, tag="h", name=f"h0_{bb}")
        x = x_ap(bb)
        for t in range(NT):
            ps = psA.tile([HID, 512], FP32, tag="psA")
            nc.tensor.matmul(ps[:], we_ap, x[:, t * 512:(t + 1) * 512],
                             start=True, stop=True)
            nc.scalar.copy(h0[:, t * 512:(t + 1) * 512], ps[:])
        hs[bb] = h0

    def make_msg_tile(bb, i):
        msgs[bb] = msg_pool.tile([128, NC8, HID], FP8, tag="msg",
                                 name=f"msg{bb}_{i}")

    def emit_msg_half(bb, i, half):
        # msg[n, k] = h @ Wl[i]; 4 node-chunks of 128 share one PSUM bank,
        # one DVE op adds the (4x-tiled) bias and casts to fp8.
        h, msg_t = hs[bb], msgs[bb]
        pm = psM.tile([128, 4 * HID], FP32, tag="psM")
        for j in range(4):
            c = 4 * half + j
            nc.tensor.matmul(pm[:, j * HID:(j + 1) * HID],
                             h[:, c * 128:(c + 1) * 128], wl_ap[i],
                             start=True, stop=True)
        nc.vector.tensor_add(msg_t[:, 4 * half:4 * half + 4, :], pm[:], bl_t[i][:])

    def emit_agg(bb, i, t, h_new, msg_t, relu_split=False):
        # aggT[k, n-tile] = msg.T @ adjT via 4 fp8 DoubleRow matmuls
        # (each contracts 2 node-chunks = 256 sources); relu + rescale.
        adj = adj_t[bb][t]
        ps = psA.tile([HID, 512], FP32, tag="psA")
        for c in range(4):
            nc.tensor.matmul(ps[:], msg_t[:, 2 * c:2 * c + 2, :],
                             adj[:, 2 * c:2 * c + 2, :],
                             start=(c == 0), stop=(c == 3), perf_mode=DR)
        lo = t * 512
        if relu_split:  # tail: release h in quarters so proj starts sooner
            nc.scalar.activation(h_new[:, lo:lo + 256], ps[:, 0:256], RELU,
                                 scale=S[i] / S[i + 1])
            nc.scalar.activation(h_new[:, lo + 256:lo + 512], ps[:, 256:512],
                                 RELU, scale=S[i] / S[i + 1])
        else:
            nc.scalar.activation(h_new[:, lo:lo + 512], ps[:], RELU,
                                 scale=S[i] / S[i + 1])

    o_ts = [None] * BPC

    def emit_proj_half(bb, t):
        if o_ts[bb] is None:
            o_ts[bb] = o_pool.tile([OUT, N], FP16, tag="o", name=f"o{bb}")
        o_t = o_ts[bb]
        po = psO.tile([OUT, 512], FP32, tag="psO")
        nc.tensor.matmul(po[:], wp_ap, hs[bb][:, t * 512:(t + 1) * 512],
                         start=True, stop=True)
        lo = t * 512
        nc.vector.tensor_scalar_add(o_t[:, lo:lo + 512], po[:], 0.0)
        nc.gpsimd.dma_start(out[bb, :, lo:lo + 512], o_t[:, lo:lo + 512])

    def emit_proj_q(bb, t, q, po):
        # tail-exposed quarter: MM [64, 256] into a shared PSUM bank,
        # ACT/DVE alternate the copy, store triggers immediately so the
        # final DMA's completion semaphore (2-4us lag) starts early.
        if o_ts[bb] is None:
            o_ts[bb] = o_pool.tile([OUT, N], FP16, tag="o", name=f"o{bb}")
        o_t = o_ts[bb]
        lo = t * 512 + q * 256
        nc.tensor.matmul(po[:, q * 256:(q + 1) * 256], wp_ap,
                         hs[bb][:, lo:lo + 256], start=True, stop=True)
        if q == 0:
            nc.scalar.copy(o_t[:, lo:lo + 256], po[:, 0:256])
        else:
            nc.vector.tensor_scalar_add(o_t[:, lo:lo + 256], po[:, 256:512], 0.0)
        nc.gpsimd.dma_start(out[bb, :, lo:lo + 256], o_t[:, lo:lo + 256])

    # ---- prologue: embed + msg-layer-0 for pair (0,1) ----------------
    # (pair (2,3)'s embed/msg0 and the later bias broadcasts interleave
    # into pair (0,1)'s rounds as `extra` units.)
    emit_embed(0)
    emit_embed(1)
    emit_bias_bcast(0)
    for bb in (0, 1):
        make_msg_tile(bb, 0)
        emit_msg_half(bb, 0, 0)
        emit_msg_half(bb, 0, 1)

    # ---- paired rounds ------------------------------------------------
    # Round (A, B, layer i): A/B agg groups alternate so each graph's
    # relu -> msg matmul -> DVE-quantize chain hides under the other
    # graph's DoubleRow aggregations. On the last layer the msg slots
    # carry the projection instead. `extra` units (next pair's
    # embed/msg0, bias broadcasts) fill the remaining slack.
    def round_(A, BQ, i, extra, split_tail=False):
        ex = list(extra)

        def drain(k):
            for _ in range(k):
                if ex:
                    ex.pop(0)()

        hA = h_pool.tile([HID, N], FP16, tag="h", name=f"h{i + 1}_{A}")
        hB = h_pool.tile([HID, N], FP16, tag="h", name=f"h{i + 1}_{BQ}")
        msgA, msgB = msgs[A], msgs[BQ]
        last = i == NUM_LAYERS - 1
        emit_agg(A, i, 0, hA, msgA)
        drain(1)
        emit_agg(BQ, i, 0, hB, msgB)
        hs[A] = hA
        if not last:
            make_msg_tile(A, i + 1)
            emit_msg_half(A, i + 1, 0)  # needs relu(A, t0): done during B.t0
        else:
            emit_proj_half(A, 0)
        emit_agg(A, i, 1, hA, msgA)
        drain(1)
        hs[BQ] = hB
        if not last:
            make_msg_tile(BQ, i + 1)
            emit_msg_half(BQ, i + 1, 0)
        else:
            emit_proj_half(BQ, 0)
        emit_agg(BQ, i, 1, hB, msgB, relu_split=split_tail)
        if not last:
            emit_msg_half(A, i + 1, 1)
            drain(1)
            emit_msg_half(BQ, i + 1, 1)
        elif split_tail:
            emit_proj_half(A, 1)
            poq = psO.tile([OUT, 512], FP32, tag="psO")
            emit_proj_q(BQ, 1, 0, poq)
            emit_proj_q(BQ, 1, 1, poq)
        else:
            emit_proj_half(A, 1)
            emit_proj_half(BQ, 1)
        drain(len(ex))

    def units_embed_msg0(bb):
        def u1():
            emit_embed(bb)

        def u2():
            make_msg_tile(bb, 0)
            emit_msg_half(bb, 0, 0)
            emit_msg_half(bb, 0, 1)

        return [u1, u2]

    round_(0, 1, 0, [lambda: emit_bias_bcast(1)] + units_embed_msg0(2))
    round_(0, 1, 1, [lambda: emit_bias_bcast(2)] + units_embed_msg0(3))
    round_(0, 1, 2, [])
    round_(2, 3, 0, [])
    round_(2, 3, 1, [])
    round_(2, 3, 2, [], split_tail=True)


def build_nc():
    # Bacc (not raw Bass): its compile() runs generate_event_semaphores,
    # which splits multi-sem waits down to the 1-wait-per-instruction
    # hardware limit walrus enforces.
    nc = bacc.Bacc("TRN2", debug=False, num_devices=N_CORES, num_swdge_queues=2)
    adjT8 = nc.dram_tensor("adjT8", [BPC, NT, 128, NC8, 512], FP8,
                           kind="ExternalInput").ap()
    packA = nc.dram_tensor("packA", [64, PACKA_W], FP16,
                           kind="ExternalInput").ap()
    packB = nc.dram_tensor("packB", [128, PACKB_W], FP16,
                           kind="ExternalInput").ap()
    xT = nc.dram_tensor("xT", [BPC, IN_DIM, N], FP16, kind="ExternalInput").ap()
    out = nc.dram_tensor("out", [BPC, OUT, N], FP16, kind="ExternalOutput").ap()

    with tile.TileContext(nc) as tc, ExitStack() as ctx:
        _kernel_body(ctx, tc, out, adjT8, packA, packB, xT)
    nc.compile()
    return nc


def make_in_maps(node_features, adjacency_matrix, node_mask, W_embed, Wl, bl,
                 W_proj, b_proj):
    x = np.asarray(node_features, dtype=np.float32)
    adj = np.asarray(adjacency_matrix, dtype=np.float32)
    bl4 = np.tile(
        (np.asarray(bl, np.float64) * BLZ / np.array(S[:NUM_LAYERS])[:, None]),
        (1, 4))
    in_maps = []
    for c in range(N_CORES):
        sl = slice(c * BPC, (c + 1) * BPC)
        xTc = np.ascontiguousarray(x[sl].transpose(0, 2, 1)).astype(np.float16)
        pka = np.zeros((64, PACKA_W), np.float16)
        pka[:, PKA_WE:PKA_WE + HID] = np.asarray(W_embed, np.float16)
        pka[:, PKA_X0:PKA_X0 + N] = xTc[0]
        pkb = np.zeros((128, PACKB_W), np.float16)
        for i in range(NUM_LAYERS):
            pkb[:, PKB_WL + HID * i:PKB_WL + HID * (i + 1)] = np.asarray(
                Wl[i], np.float16)
            pkb[0, PKB_BL + 512 * i:PKB_BL + 512 * (i + 1)] = bl4[i].astype(
                np.float16)
        pkb[:, PKB_WP:PKB_WP + OUT] = np.asarray(W_proj, np.float16)
        pkb[0:64, PKB_X1:PKB_X1 + N] = xTc[1]
        # adjT8[bb, t, p, c, j] = adj[bb, t*512+j, c*128+p]
        a = adj[sl].reshape(BPC, NT, 512, NC8, 128).transpose(0, 1, 4, 3, 2)
        in_maps.append({
            "adjT8": np.ascontiguousarray(a).astype(ml_dtypes.float8_e4m3fn),
            "packA": pka,
            "packB": pkb,
            "xT": xTc,
        })
    return in_maps


_NC_CACHE = None


def get_nc():
    global _NC_CACHE
    if _NC_CACHE is None:
        _NC_CACHE = build_nc()
    return _NC_CACHE


def kernel(**inputs):
    nc = get_nc()
    in_maps = make_in_maps(**inputs)
    res = run_bass_kernel_spmd(nc, in_maps, list(range(N_CORES)))
    outs = [np.asarray(res.results[c]["out"]) for c in range(N_CORES)]
    oT = np.concatenate(outs, axis=0).astype(np.float32)  # [B, OUT, N]
    o = oT.transpose(0, 2, 1) * np.float32(S[NUM_LAYERS])
    o = o + np.asarray(inputs["b_proj"], np.float32)
    o = o * np.asarray(inputs["node_mask"], np.float32)
    return o


if __name__ == "__main__":
    rng = np.random.default_rng(0)
    ins = {
        "node_features": rng.standard_normal((B, N, IN_DIM), dtype=np.float32),
        "adjacency_matrix": rng.random((B, N, N), dtype=np.float32),
        "node_mask": np.ones((B, N, 1), np.float32),
        "W_embed": rng.standard_normal((IN_DIM, HID), dtype=np.float32) * 0.1,
        "Wl": rng.standard_normal((NUM_LAYERS, HID, HID), dtype=np.float32) * 0.08,
        "bl": rng.standard_normal((NUM_LAYERS, HID), dtype=np.float32) * 0.08,
        "W_proj": rng.standard_normal((HID, 2 * 32), dtype=np.float32) * 0.08,
        "b_proj": rng.standard_normal((2 * 32,), dtype=np.float32) * 0.08,
    }
    out = kernel(**ins)
    print("out", out.shape, out.dtype, float(np.abs(out).mean()))


# revision 14
# speedup vs baseline: 1.2069x; 1.0363x over previous
"""Bass/Trainium2 kernel for nn_EuclideanGraphEncoder (GCN message passing).

Strategy: data-parallel over the batch (4 graphs per core, 8 cores),
weights replicated, no collectives. The dominant aggregation matmul
(adj @ msg) runs in fp8-e4m3 DoubleRow mode (2 k-tiles per instruction,
2x fp8 throughput): the adjacency is downcast+transposed to fp8 on the
host, and the per-layer messages are quantized to fp8 on the DVE during
the PSUM->SBUF bias-add. Node features h stay fp16 (their quantization
error would hit the output un-averaged), so the final rel-err keeps the
adjacency/message fp8 noise only, which the 1024-wide sum-aggregation
averages down to ~1.5e-2.

Device-side layout: h is kept transposed [hid=128 partitions, n=1024]
fp16. Per layer:
  msg[n,k]  = h @ Wl          (8x K=128 matmuls; DVE adds bias, casts fp8)
  aggT[k,n] = msg.T @ adjT    (2 n-tiles x 4 DoubleRow fp8 matmuls)
  hT        = relu(aggT)      (ACT, PSUM->SBUF fp16, power-of-2 rescale)
Projection keeps the transposed layout (W_proj stationary, hT moving,
N=512) and returns outT [64, n] fp16 scaled by 1/S3; the host transposes,
rescales, adds b_proj and applies the node mask.

Scheduling notes (from NTFF profiles): DMA completion semaphores lag
their bytes by 2-4us each during the ring ramp and serialize per queue,
so every small tensor the prologue needs ships as ONE packed transfer
(one semaphore); full-array dependency-free matmuls spin the PE through
the HAM clock ramp (1.2 -> 2.4 GHz, ~3.4us of sustained full-array
activity -- rank-1 warm-ups do NOT trip the monitor) while the rings
spin up; the per-layer biases ride the pack as [1, 512] rows and are
partition-broadcast by a rank-1 fp16 matmul; graphs run in
software-pipelined pairs so each graph's relu -> msg -> fp8-quantize
chain hides under the other graph's DoubleRow aggregations; out stores
trigger from SWDGE so the ACT engine never stalls on descriptor
generation.
"""

import sys
from contextlib import ExitStack

import ml_dtypes
import numpy as np

try:
    import concourse.bass as bass
except ImportError:  # fall back to the repo checkout
    sys.path.insert(0, "/opt/trn_rl_repo")
    import concourse.bass as bass

import concourse.tile as tile
from concourse import bacc, mybir
from concourse.bass_utils import run_bass_kernel_spmd

B, N, IN_DIM, HID, OUT = 32, 1024, 64, 128, 64
NUM_LAYERS = 3
N_CORES = 8
BPC = B // N_CORES  # graphs per core
NT = N // 512  # aggregation free-dim tiles
NC8 = N // 128  # node chunks of 128

FP8 = mybir.dt.float8e4
FP16 = mybir.dt.float16
FP32 = mybir.dt.float32
RELU = mybir.ActivationFunctionType.Relu
DR = mybir.MatmulPerfMode.DoubleRow

# Per-layer power-of-2 scales: SBUF h/msg tiles hold h_true / S[i] so fp16
# never overflows (true agg magnitudes reach ~5e6). Scale hops are exact
# (powers of two) and ride the relu ACT scale; the host pre-scales the
# biases and post-scales the output.
S = [1.0, 64.0, 16384.0, 4194304.0]
BLZ = 2.0 ** 14  # bias rows pre-scaled so fp16 never subnormalizes

# pack layouts (fp16 columns): the small tensors the prologue depends on
# ship as TWO packed transfers -- one per HWDGE ring -- so two completion
# semaphores (which lag their bytes by 2-4us during the ring ramp and
# serialize per queue) gate everything in parallel.
# pack_a (sync ring, [64, *]): W_embed + x of graph 0.
PKA_WE = 0
PKA_X0 = HID
PACKA_W = PKA_X0 + N
# pack_b (scalar ring, [128, *]): Wl, W_proj, bias rows, x of graph 1.
PKB_WL = 0                  # [0:128, 128i:128(i+1)]
PKB_WP = 3 * HID            # [0:128, 384:448]
PKB_BL = PKB_WP + OUT       # [0:1,   448+512i]  (x4 tiled, x BLZ)
PKB_X1 = PKB_BL + 3 * 512   # [0:64,  1984:3008]
PACKB_W = PKB_X1 + N

WARM_MMS = 10  # PE clock warm-up matmuls covering the DMA-ring spin-up


def _kernel_body(ctx, tc, out, adjT8, packA, packB, xT):
    nc = tc.nc

    consts = ctx.enter_context(tc.tile_pool(name="consts", bufs=1))
    adj_pool = ctx.enter_context(tc.tile_pool(name="adj", bufs=BPC * NT))
    xt_pool = ctx.enter_context(tc.tile_pool(name="xt", bufs=BPC - 1))
    h_pool = ctx.enter_context(tc.tile_pool(name="h", bufs=8))
    msg_pool = ctx.enter_context(tc.tile_pool(name="msg", bufs=6))
    o_pool = ctx.enter_context(tc.tile_pool(name="o", bufs=BPC))
    psA = ctx.enter_context(tc.tile_pool(name="psA", bufs=3, space="PSUM"))
    psM = ctx.enter_context(tc.tile_pool(name="psM", bufs=3, space="PSUM"))
    psO = ctx.enter_context(tc.tile_pool(name="psO", bufs=2, space="PSUM"))

    # ---- PE clock pre-warm -------------------------------------------
    # Dependency-free FULL-ARRAY matmuls from t=0: the HAM un-throttles
    # (1.2 -> 2.4 GHz) after ~3.4us of sustained PE activity, so the real
    # work (gated on the first DMA deliveries at ~9us) starts at full
    # clock. K=1 warm-ups don't register on the activity monitor.
    warm_w = consts.tile([128, HID], FP16, tag="warm_w")
    warm_m = consts.tile([128, 512], FP16, tag="warm_m")
    nc.vector.memset(warm_w[:], 0.7071)
    # varied column bands: the moving operand must toggle the PE datapath
    # cycle-to-cycle or the activity monitor won't count the work.
    for j, v in enumerate([1.0, -777.0, 0.013, 3e4, -2.0e-4, 255.0, -0.5, 41.0]):
        nc.vector.memset(warm_m[:, j * 64:(j + 1) * 64], v)
    ones_h = consts.tile([1, HID], FP16, tag="ones_h")
    nc.vector.memset(ones_h[:], 1.0)
    for w in range(WARM_MMS):
        psw = psA.tile([HID, 512], FP32, tag="psA", name="psw")
        nc.tensor.matmul(psw[:], warm_w[:], warm_m[:], start=True, stop=True)

    # ---- loads --------------------------------------------------------
    # Sync HWDGE ring: the pack, then the adj flood. The second HWDGE
    # ring (scalar) carries x1 in parallel; x2/x3 ride SWDGE (gpsimd),
    # which also triggers the out stores.
    pka_t = consts.tile([64, PACKA_W], FP16, tag="pka")
    nc.sync.dma_start(pka_t[:], packA[:, :])
    pkb_t = consts.tile([128, PACKB_W], FP16, tag="pkb")
    nc.scalar.dma_start(pkb_t[:], packB[:, :])

    xts = [None, None] + [xt_pool.tile([IN_DIM, N], FP16, tag="xt",
                                       name=f"xt{bb}") for bb in (2, 3)]
    for bb in (2, 3):
        nc.gpsimd.dma_start(xts[bb][:], xT[bb])

    def x_ap(bb):
        if bb == 0:
            return pka_t[0:64, PKA_X0:PKA_X0 + N]
        if bb == 1:
            return pkb_t[0:64, PKB_X1:PKB_X1 + N]
        return xts[bb][:]

    we_ap = pka_t[0:64, PKA_WE:PKA_WE + HID]
    wl_ap = [pkb_t[:, PKB_WL + HID * i:PKB_WL + HID * (i + 1)]
             for i in range(NUM_LAYERS)]
    wp_ap = pkb_t[:, PKB_WP:PKB_WP + OUT]
    blr_ap = [pkb_t[0:1, PKB_BL + 512 * i:PKB_BL + 512 * (i + 1)]
              for i in range(NUM_LAYERS)]
    bl_t = [consts.tile([128, 4 * HID], FP32, tag=f"bl{i}", name=f"bl{i}")
            for i in range(NUM_LAYERS)]

    # adj: one 512KB DMA per (graph, n-half); 4KB contiguous per partition.
    adj_t = [[adj_pool.tile([128, NC8, 512], FP8, tag="adj",
                            name=f"adj{bb}_{t}") for t in range(NT)]
             for bb in range(BPC)]
    nc.sync.dma_start(adj_t[0][0][:, 0:4, :], adjT8[0, 0, :, 0:4, :])
    nc.sync.dma_start(adj_t[0][0][:, 4:8, :], adjT8[0, 0, :, 4:8, :])
    for bb, t in [(1, 0), (0, 1), (1, 1), (2, 0), (3, 0), (2, 1), (3, 1)]:
        nc.sync.dma_start(adj_t[bb][t][:], adjT8[bb, t])

    # ---- emission helpers --------------------------------------------
    hs = [None] * BPC    # current hT tile per graph
    msgs = [None] * BPC  # current msg tile per graph

    def emit_bias_bcast(i):
        # bias row -> all 128 partitions via a rank-1 fp16 matmul; the row
        # is pre-scaled by 2^14 so fp16 never subnormalizes, and the ACT
        # copy undoes the (exact power-of-2) factor.
        pb = psM.tile([128, 4 * HID], FP32, tag="psM", name=f"pb{i}")
        nc.tensor.matmul(pb[:], ones_h[:], blr_ap[i], start=True, stop=True)
        nc.scalar.mul(bl_t[i][:], pb[:], 1.0 / BLZ)

    def emit_embed(bb):
        h0 = h_pool.tile([HID, N], FP16, tag="h", name=f"h0_{bb}")
        x = x_ap(bb)
        for t in range(NT):
            ps = psA.tile([HID, 512], FP32, tag="psA")
            nc.tensor.matmul(ps[:], we_ap, x[:, t * 512:(t + 1) * 512],
                             start=True, stop=True)
            nc.scalar.copy(h0[:, t * 512:(t + 1) * 512], ps[:])
        hs[bb] = h0

    def make_msg_tile(bb, i):
        msgs[bb] = msg_pool.tile([128, NC8, HID], FP8, tag="msg",
                                 name=f"msg{bb}_{i}")

    def emit_msg_half(bb, i, half):
        # msg[n, k] = h @ Wl[i]; 4 node-chunks of 128 share one PSUM bank,
        # one DVE op adds the (4x-tiled) bias and casts to fp8.
        h, msg_t = hs[bb], msgs[bb]
        pm = psM.tile([128, 4 * HID], FP32, tag="psM")
        for j in range(4):
            c = 4 * half + j
            nc.tensor.matmul(pm[:, j * HID:(j + 1) * HID],
                             h[:, c * 128:(c + 1) * 128], wl_ap[i],
                             start=True, stop=True)
        nc.vector.tensor_add(msg_t[:, 4 * half:4 * half + 4, :], pm[:], bl_t[i][:])

    def emit_agg(bb, i, t, h_new, msg_t, relu_split=False):
        # aggT[k, n-tile] = msg.T @ adjT via 4 fp8 DoubleRow matmuls
        # (each contracts 2 node-chunks = 256 sources); relu + rescale.
        adj = adj_t[bb][t]
        ps = psA.tile([HID, 512], FP32, tag="psA")
        for c in range(4):
            nc.tensor.matmul(ps[:], msg_t[:, 2 * c:2 * c + 2, :],
                             adj[:, 2 * c:2 * c + 2, :],
                             start=(c == 0), stop=(c == 3), perf_mode=DR)
        lo = t * 512
        if relu_split:  # tail: release h in quarters so proj starts sooner
            nc.scalar.activation(h_new[:, lo:lo + 256], ps[:, 0:256], RELU,
                                 scale=S[i] / S[i + 1])
            nc.scalar.activation(h_new[:, lo + 256:lo + 512], ps[:, 256:512],
                                 RELU, scale=S[i] / S[i + 1])
        else:
            nc.scalar.activation(h_new[:, lo:lo + 512], ps[:], RELU,
                                 scale=S[i] / S[i + 1])

    o_ts = [None] * BPC

    def emit_proj_half(bb, t):
        if o_ts[bb] is None:
            o_ts[bb] = o_pool.tile([OUT, N], FP16, tag="o", name=f"o{bb}")
        o_t = o_ts[bb]
        po = psO.tile([OUT, 512], FP32, tag="psO")
        nc.tensor.matmul(po[:], wp_ap, hs[bb][:, t * 512:(t + 1) * 512],
                         start=True, stop=True)
        lo = t * 512
        nc.vector.tensor_scalar_add(o_t[:, lo:lo + 512], po[:], 0.0)
        nc.sync.dma_start(out[bb, :, lo:lo + 512], o_t[:, lo:lo + 512])

    def emit_proj_q(bb, t, q, po):
        # tail-exposed quarter: MM [64, 256] into a shared PSUM bank,
        # ACT/DVE alternate the copy, store triggers immediately so the
        # final DMA's completion semaphore (2-4us lag) starts early.
        if o_ts[bb] is None:
            o_ts[bb] = o_pool.tile([OUT, N], FP16, tag="o", name=f"o{bb}")
        o_t = o_ts[bb]
        lo = t * 512 + q * 256
        nc.tensor.matmul(po[:, q * 256:(q + 1) * 256], wp_ap,
                         hs[bb][:, lo:lo + 256], start=True, stop=True)
        if q == 0:
            nc.scalar.copy(o_t[:, lo:lo + 256], po[:, 0:256])
        else:
            nc.vector.tensor_scalar_add(o_t[:, lo:lo + 256], po[:, 256:512], 0.0)
        nc.sync.dma_start(out[bb, :, lo:lo + 256], o_t[:, lo:lo + 256])

    # ---- prologue: embed + msg-layer-0 for pair (0,1) ----------------
    # (pair (2,3)'s embed/msg0 and the later bias broadcasts interleave
    # into pair (0,1)'s rounds as `extra` units.)
    emit_embed(0)
    emit_embed(1)
    emit_bias_bcast(0)
    for bb in (0, 1):
        make_msg_tile(bb, 0)
        emit_msg_half(bb, 0, 0)
        emit_msg_half(bb, 0, 1)

    for w in range(2):
        psw = psA.tile([HID, 512], FP32, tag="psA", name="psw2")
        nc.tensor.matmul(psw[:], warm_w[:], warm_m[:], start=True, stop=True)

    # ---- paired rounds ------------------------------------------------
    # Round (A, B, layer i): A/B agg groups alternate so each graph's
    # relu -> msg matmul -> DVE-quantize chain hides under the other
    # graph's DoubleRow aggregations. On the last layer the msg slots
    # carry the projection instead. `extra` units (next pair's
    # embed/msg0, bias broadcasts) fill the remaining slack.
    def round_(A, BQ, i, extra, split_tail=False):
        ex = list(extra)

        def drain(k):
            for _ in range(k):
                if ex:
                    ex.pop(0)()

        hA = h_pool.tile([HID, N], FP16, tag="h", name=f"h{i + 1}_{A}")
        hB = h_pool.tile([HID, N], FP16, tag="h", name=f"h{i + 1}_{BQ}")
        msgA, msgB = msgs[A], msgs[BQ]
        last = i == NUM_LAYERS - 1
        emit_agg(A, i, 0, hA, msgA)
        drain(1)
        emit_agg(BQ, i, 0, hB, msgB)
        hs[A] = hA
        if not last:
            make_msg_tile(A, i + 1)
            emit_msg_half(A, i + 1, 0)  # needs relu(A, t0): done during B.t0
        else:
            emit_proj_half(A, 0)
        emit_agg(A, i, 1, hA, msgA)
        drain(1)
        hs[BQ] = hB
        if not last:
            make_msg_tile(BQ, i + 1)
            emit_msg_half(BQ, i + 1, 0)
        else:
            emit_proj_half(BQ, 0)
        emit_agg(BQ, i, 1, hB, msgB, relu_split=split_tail)
        if not last:
            emit_msg_half(A, i + 1, 1)
            drain(1)
            emit_msg_half(BQ, i + 1, 1)
        elif split_tail:
            emit_proj_half(A, 1)
            poq = psO.tile([OUT, 512], FP32, tag="psO")
            emit_proj_q(BQ, 1, 0, poq)
            emit_proj_q(BQ, 1, 1, poq)
        else:
            emit_proj_half(A, 1)
            emit_proj_half(BQ, 1)
        drain(len(ex))

    def units_embed_msg0(bb):
        def u1():
            emit_embed(bb)

        def u2():
            make_msg_tile(bb, 0)
            emit_msg_half(bb, 0, 0)
            emit_msg_half(bb, 0, 1)

        return [u1, u2]

    round_(0, 1, 0, [lambda: emit_bias_bcast(1)] + units_embed_msg0(2))
    round_(0, 1, 1, [lambda: emit_bias_bcast(2)] + units_embed_msg0(3))
    round_(0, 1, 2, [])
    round_(2, 3, 0, [])
    round_(2, 3, 1, [])
    round_(2, 3, 2, [], split_tail=True)


def build_nc():
    # Bacc (not raw Bass): its compile() runs generate_event_semaphores,
    # which splits multi-sem waits down to the 1-wait-per-instruction
    # hardware limit walrus enforces.
    nc = bacc.Bacc("TRN2", debug=False, num_devices=N_CORES, num_swdge_queues=2)
    adjT8 = nc.dram_tensor("adjT8", [BPC, NT, 128, NC8, 512], FP8,
                           kind="ExternalInput").ap()
    packA = nc.dram_tensor("packA", [64, PACKA_W], FP16,
                           kind="ExternalInput").ap()
    packB = nc.dram_tensor("packB", [128, PACKB_W], FP16,
                           kind="ExternalInput").ap()
    xT = nc.dram_tensor("xT", [BPC, IN_DIM, N], FP16, kind="ExternalInput").ap()
    out = nc.dram_tensor("out", [BPC, OUT, N], FP16, kind="ExternalOutput").ap()

    with tile.TileContext(nc) as tc, ExitStack() as ctx:
        _kernel_body(ctx, tc, out, adjT8, packA, packB, xT)
    nc.compile()
    return nc


def make_in_maps(node_features, adjacency_matrix, node_mask, W_embed, Wl, bl,
                 W_proj, b_proj):
    x = np.asarray(node_features, dtype=np.float32)
    adj = np.asarray(adjacency_matrix, dtype=np.float32)
    bl4 = np.tile(
        (np.asarray(bl, np.float64) * BLZ / np.array(S[:NUM_LAYERS])[:, None]),
        (1, 4))
    in_maps = []
    for c in range(N_CORES):
        sl = slice(c * BPC, (c + 1) * BPC)
        xTc = np.ascontiguousarray(x[sl].transpose(0, 2, 1)).astype(np.float16)
        pka = np.zeros((64, PACKA_W), np.float16)
        pka[:, PKA_WE:PKA_WE + HID] = np.asarray(W_embed, np.float16)
        pka[:, PKA_X0:PKA_X0 + N] = xTc[0]
        pkb = np.zeros((128, PACKB_W), np.float16)
        for i in range(NUM_LAYERS):
            pkb[:, PKB_WL + HID * i:PKB_WL + HID * (i + 1)] = np.asarray(
                Wl[i], np.float16)
            pkb[0, PKB_BL + 512 * i:PKB_BL + 512 * (i + 1)] = bl4[i].astype(
                np.float16)
        pkb[:, PKB_WP:PKB_WP + OUT] = np.asarray(W_proj, np.float16)
        pkb[0:64, PKB_X1:PKB_X1 + N] = xTc[1]
        # adjT8[bb, t, p, c, j] = adj[bb, t*512+j, c*128+p]
        a = adj[sl].reshape(BPC, NT, 512, NC8, 128).transpose(0, 1, 4, 3, 2)
        in_maps.append({
            "adjT8": np.ascontiguousarray(a).astype(ml_dtypes.float8_e4m3fn),
            "packA": pka,
            "packB": pkb,
            "xT": xTc,
        })
    return in_maps


_NC_CACHE = None


def get_nc():
    global _NC_CACHE
    if _NC_CACHE is None:
        _NC_CACHE = build_nc()
    return _NC_CACHE


def kernel(**inputs):
    nc = get_nc()
    in_maps = make_in_maps(**inputs)
    res = run_bass_kernel_spmd(nc, in_maps, list(range(N_CORES)))
    outs = [np.asarray(res.results[c]["out"]) for c in range(N_CORES)]
    oT = np.concatenate(outs, axis=0).astype(np.float32)  # [B, OUT, N]
    o = oT.transpose(0, 2, 1) * np.float32(S[NUM_LAYERS])
    o = o + np.asarray(inputs["b_proj"], np.float32)
    o = o * np.asarray(inputs["node_mask"], np.float32)
    return o


if __name__ == "__main__":
    rng = np.random.default_rng(0)
    ins = {
        "node_features": rng.standard_normal((B, N, IN_DIM), dtype=np.float32),
        "adjacency_matrix": rng.random((B, N, N), dtype=np.float32),
        "node_mask": np.ones((B, N, 1), np.float32),
        "W_embed": rng.standard_normal((IN_DIM, HID), dtype=np.float32) * 0.1,
        "Wl": rng.standard_normal((NUM_LAYERS, HID, HID), dtype=np.float32) * 0.08,
        "bl": rng.standard_normal((NUM_LAYERS, HID), dtype=np.float32) * 0.08,
        "W_proj": rng.standard_normal((HID, 2 * 32), dtype=np.float32) * 0.08,
        "b_proj": rng.standard_normal((2 * 32,), dtype=np.float32) * 0.08,
    }
    out = kernel(**ins)
    print("out", out.shape, out.dtype, float(np.abs(out).mean()))


# revision 17
# speedup vs baseline: 1.2189x; 1.0099x over previous
"""Bass/Trainium2 kernel for nn_EuclideanGraphEncoder (GCN message passing).

Strategy: data-parallel over the batch (4 graphs per core, 8 cores),
weights replicated, no collectives. The dominant aggregation matmul
(adj @ msg) runs in fp8-e4m3 DoubleRow mode (2 k-tiles per instruction,
2x fp8 throughput): the adjacency is downcast+transposed to fp8 on the
host, and the per-layer messages are quantized to fp8 on the DVE during
the PSUM->SBUF bias-add. Node features h stay fp16 (their quantization
error would hit the output un-averaged), so the final rel-err keeps the
adjacency/message fp8 noise only, which the 1024-wide sum-aggregation
averages down to ~1.5e-2.

Device-side layout: h is kept transposed [hid=128 partitions, n=1024]
fp16. Per layer:
  msg[n,k]  = h @ Wl          (8x K=128 matmuls; DVE adds bias, casts fp8)
  aggT[k,n] = msg.T @ adjT    (2 n-tiles x 4 DoubleRow fp8 matmuls)
  hT        = relu(aggT)      (ACT, PSUM->SBUF fp16, power-of-2 rescale)
Projection keeps the transposed layout (W_proj stationary, hT moving,
N=512) and returns outT [64, n] fp16 scaled by 1/S3; the host transposes,
rescales, adds b_proj and applies the node mask.

Scheduling notes (from NTFF profiles): DMA completion semaphores lag
their bytes by 2-4us each during the ring ramp and serialize per queue,
so every small tensor the prologue needs ships as ONE packed transfer
(one semaphore); full-array dependency-free matmuls spin the PE through
the HAM clock ramp (1.2 -> 2.4 GHz, ~3.4us of sustained full-array
activity -- rank-1 warm-ups do NOT trip the monitor) while the rings
spin up; the per-layer biases ride the pack as [1, 512] rows and are
partition-broadcast by a rank-1 fp16 matmul; graphs run in
software-pipelined pairs so each graph's relu -> msg -> fp8-quantize
chain hides under the other graph's DoubleRow aggregations; out stores
trigger from SWDGE so the ACT engine never stalls on descriptor
generation.
"""

import sys
from contextlib import ExitStack

import ml_dtypes
import numpy as np

try:
    import concourse.bass as bass
except ImportError:  # fall back to the repo checkout
    sys.path.insert(0, "/opt/trn_rl_repo")
    import concourse.bass as bass

import concourse.tile as tile
from concourse import bacc, mybir
from concourse.bass_utils import run_bass_kernel_spmd

B, N, IN_DIM, HID, OUT = 32, 1024, 64, 128, 64
NUM_LAYERS = 3
N_CORES = 8
BPC = B // N_CORES  # graphs per core
NT = N // 512  # aggregation free-dim tiles
NC8 = N // 128  # node chunks of 128

FP8 = mybir.dt.float8e4
FP16 = mybir.dt.float16
FP32 = mybir.dt.float32
RELU = mybir.ActivationFunctionType.Relu
DR = mybir.MatmulPerfMode.DoubleRow

# Per-layer power-of-2 scales: SBUF h/msg tiles hold h_true / S[i] so fp16
# never overflows (true agg magnitudes reach ~5e6). Scale hops are exact
# (powers of two) and ride the relu ACT scale; the host pre-scales the
# biases and post-scales the output.
S = [1.0, 64.0, 16384.0, 4194304.0]
BLZ = 2.0 ** 14  # bias rows pre-scaled so fp16 never subnormalizes

# pack layouts (fp16 columns): the small tensors the prologue depends on
# ship as TWO packed transfers -- one per HWDGE ring -- so two completion
# semaphores (which lag their bytes by 2-4us during the ring ramp and
# serialize per queue) gate everything in parallel.
# pack_a (sync ring, [64, *]): W_embed + x of graph 0.
PKA_WE = 0
PKA_X0 = HID
PACKA_W = PKA_X0 + N
# pack_b (scalar ring, [128, *]): Wl, W_proj, bias rows, x of graph 1.
PKB_WL = 0                  # [0:128, 128i:128(i+1)]
PKB_WP = 3 * HID            # [0:128, 384:448]
PKB_BL = PKB_WP + OUT       # [0:1,   448+512i]  (x4 tiled, x BLZ)
PKB_X1 = PKB_BL + 3 * 512   # [0:64,  1984:3008]
PACKB_W = PKB_X1 + N

WARM_MMS = 10  # PE clock warm-up matmuls covering the DMA-ring spin-up


def _kernel_body(ctx, tc, out, adjT8, packA, packB, xT):
    nc = tc.nc

    consts = ctx.enter_context(tc.tile_pool(name="consts", bufs=1))
    adj_pool = ctx.enter_context(tc.tile_pool(name="adj", bufs=BPC * NT))
    xt_pool = ctx.enter_context(tc.tile_pool(name="xt", bufs=BPC - 1))
    h_pool = ctx.enter_context(tc.tile_pool(name="h", bufs=9))
    msg_pool = ctx.enter_context(tc.tile_pool(name="msg", bufs=8))
    o_pool = ctx.enter_context(tc.tile_pool(name="o", bufs=BPC))
    psA = ctx.enter_context(tc.tile_pool(name="psA", bufs=3, space="PSUM"))
    psM = ctx.enter_context(tc.tile_pool(name="psM", bufs=3, space="PSUM"))
    psO = ctx.enter_context(tc.tile_pool(name="psO", bufs=2, space="PSUM"))

    # ---- PE clock pre-warm -------------------------------------------
    # Dependency-free FULL-ARRAY matmuls from t=0: the HAM un-throttles
    # (1.2 -> 2.4 GHz) after ~3.4us of sustained PE activity, so the real
    # work (gated on the first DMA deliveries at ~9us) starts at full
    # clock. K=1 warm-ups don't register on the activity monitor.
    warm_w = consts.tile([128, HID], FP16, tag="warm_w")
    warm_m = consts.tile([128, 512], FP16, tag="warm_m")
    nc.vector.memset(warm_w[:], 0.7071)
    # varied column bands: the moving operand must toggle the PE datapath
    # cycle-to-cycle or the activity monitor won't count the work.
    for j, v in enumerate([1.0, -777.0, 0.013, 3e4, -2.0e-4, 255.0, -0.5, 41.0]):
        nc.vector.memset(warm_m[:, j * 64:(j + 1) * 64], v)
    ones_h = consts.tile([1, HID], FP16, tag="ones_h")
    nc.vector.memset(ones_h[:], 1.0)
    for w in range(WARM_MMS):
        psw = psA.tile([HID, 512], FP32, tag="psA", name="psw")
        nc.tensor.matmul(psw[:], warm_w[:], warm_m[:], start=True, stop=True)

    # ---- loads --------------------------------------------------------
    # Sync HWDGE ring: the pack, then the adj flood. The second HWDGE
    # ring (scalar) carries x1 in parallel; x2/x3 ride SWDGE (gpsimd),
    # which also triggers the out stores.
    pka_t = consts.tile([64, PACKA_W], FP16, tag="pka")
    nc.sync.dma_start(pka_t[:], packA[:, :])
    pkb_t = consts.tile([128, PACKB_W], FP16, tag="pkb")
    nc.scalar.dma_start(pkb_t[:], packB[:, :])

    xts = [None, None] + [xt_pool.tile([IN_DIM, N], FP16, tag="xt",
                                       name=f"xt{bb}") for bb in (2, 3)]
    for bb in (2, 3):
        nc.gpsimd.dma_start(xts[bb][:], xT[bb])

    def x_ap(bb):
        if bb == 0:
            return pka_t[0:64, PKA_X0:PKA_X0 + N]
        if bb == 1:
            return pkb_t[0:64, PKB_X1:PKB_X1 + N]
        return xts[bb][:]

    we_ap = pka_t[0:64, PKA_WE:PKA_WE + HID]
    wl_ap = [pkb_t[:, PKB_WL + HID * i:PKB_WL + HID * (i + 1)]
             for i in range(NUM_LAYERS)]
    wp_ap = pkb_t[:, PKB_WP:PKB_WP + OUT]
    blr_ap = [pkb_t[0:1, PKB_BL + 512 * i:PKB_BL + 512 * (i + 1)]
              for i in range(NUM_LAYERS)]
    bl_t = [consts.tile([128, 4 * HID], FP32, tag=f"bl{i}", name=f"bl{i}")
            for i in range(NUM_LAYERS)]

    # adj: one 512KB DMA per (graph, n-half); 4KB contiguous per partition.
    adj_t = [[adj_pool.tile([128, NC8, 512], FP8, tag="adj",
                            name=f"adj{bb}_{t}") for t in range(NT)]
             for bb in range(BPC)]
    nc.sync.dma_start(adj_t[0][0][:, 0:4, :], adjT8[0, 0, :, 0:4, :])
    nc.sync.dma_start(adj_t[0][0][:, 4:8, :], adjT8[0, 0, :, 4:8, :])
    for bb, t in [(1, 0), (0, 1), (1, 1), (2, 0), (3, 0), (2, 1), (3, 1)]:
        nc.sync.dma_start(adj_t[bb][t][:], adjT8[bb, t])

    # ---- emission helpers --------------------------------------------
    # Tiles are created lazily in emission order; pool buffer cycling
    # matches liveness (per graph: h_i + h_{i+1} live, msg_i + msg_{i+1}
    # live).
    h_tiles, msg_tiles, o_ts = {}, {}, [None] * BPC

    def get_h(bb, i):
        if (bb, i) not in h_tiles:
            h_tiles[(bb, i)] = h_pool.tile([HID, N], FP16, tag="h",
                                           name=f"h{i}_{bb}")
        return h_tiles[(bb, i)]

    def get_msg(bb, i):
        if (bb, i) not in msg_tiles:
            msg_tiles[(bb, i)] = msg_pool.tile([128, NC8, HID], FP8,
                                               tag="msg", name=f"msg{bb}_{i}")
        return msg_tiles[(bb, i)]

    def emit_bias_bcast(i):
        # bias row -> all 128 partitions via a rank-1 fp16 matmul; the row
        # is pre-scaled by 2^14 so fp16 never subnormalizes, and the ACT
        # copy undoes the (exact power-of-2) factor.
        pb = psM.tile([128, 4 * HID], FP32, tag="psM", name=f"pb{i}")
        nc.tensor.matmul(pb[:], ones_h[:], blr_ap[i], start=True, stop=True)
        nc.scalar.mul(bl_t[i][:], pb[:], 1.0 / BLZ)

    def emit_embed_t(bb, t):
        h0 = get_h(bb, 0)
        ps = psM.tile([128, 4 * HID], FP32, tag="psM", name=f"pe{bb}_{t}")
        nc.tensor.matmul(ps[:, 0:512], we_ap,
                         x_ap(bb)[:, t * 512:(t + 1) * 512],
                         start=True, stop=True)
        nc.scalar.copy(h0[:, t * 512:(t + 1) * 512], ps[:, 0:512])

    def burst_msg(bb, i, half):
        # msg[n, k] = h @ Wl[i] for 4 node-chunks, as 4 single-matmul
        # pieces (interleaved between DoubleRow matmuls so their
        # LDWEIGHTS hide); the last piece adds the bias and casts to fp8
        # on the DVE.
        st = {}

        def piece(j):
            def f():
                if "pm" not in st:
                    st["pm"] = psM.tile([128, 4 * HID], FP32, tag="psM",
                                        name=f"pm{bb}_{i}_{half}")
                c = 4 * half + j
                nc.tensor.matmul(st["pm"][:, j * HID:(j + 1) * HID],
                                 get_h(bb, i)[:, c * 128:(c + 1) * 128],
                                 wl_ap[i], start=True, stop=True)
                if j == 3:
                    nc.vector.tensor_add(
                        get_msg(bb, i)[:, 4 * half:4 * half + 4, :],
                        st["pm"][:], bl_t[i][:])
            return f

        return [piece(j) for j in range(4)]

    def emit_proj_half(bb, t):
        if o_ts[bb] is None:
            o_ts[bb] = o_pool.tile([OUT, N], FP16, tag="o", name=f"o{bb}")
        o_t = o_ts[bb]
        po = psO.tile([OUT, 512], FP32, tag="psO", name=f"po{bb}_{t}")
        lo = t * 512
        nc.tensor.matmul(po[:], wp_ap, get_h(bb, NUM_LAYERS)[:, lo:lo + 512],
                         start=True, stop=True)
        nc.vector.tensor_scalar_add(o_t[:, lo:lo + 512], po[:], 0.0)
        nc.sync.dma_start(out[bb, :, lo:lo + 512], o_t[:, lo:lo + 512])

    def emit_proj_q(bb, t, q, po):
        # tail-exposed quarter: MM [64, 256] into a shared PSUM bank,
        # ACT/DVE alternate the copy, store triggers immediately so the
        # final DMA's completion semaphore (2-4us lag) starts early.
        if o_ts[bb] is None:
            o_ts[bb] = o_pool.tile([OUT, N], FP16, tag="o", name=f"o{bb}")
        o_t = o_ts[bb]
        lo = t * 512 + q * 256
        nc.tensor.matmul(po[:, q * 256:(q + 1) * 256], wp_ap,
                         get_h(bb, NUM_LAYERS)[:, lo:lo + 256],
                         start=True, stop=True)
        if q == 0:
            nc.scalar.copy(o_t[:, lo:lo + 256], po[:, 0:256])
        else:
            nc.vector.tensor_scalar_add(o_t[:, lo:lo + 256], po[:, 256:512],
                                        0.0)
        nc.sync.dma_start(out[bb, :, lo:lo + 256], o_t[:, lo:lo + 256])

    def emit_unit(bb, i, t, comp=(), relu_split=False):
        # One aggregation unit: 4 fp8 DoubleRow matmuls (each contracts 2
        # node-chunks = 256 sources) with companion small matmuls woven
        # between them (their LDWEIGHTS hide under the 213ns DR stream),
        # then relu + power-of-2 rescale.
        msg_t, adj = get_msg(bb, i), adj_t[bb][t]
        h_new = get_h(bb, i + 1)
        ps = psA.tile([HID, 512], FP32, tag="psA", name=f"pa{bb}_{i}_{t}")
        comp = list(comp)
        for c in range(4):
            nc.tensor.matmul(ps[:], msg_t[:, 2 * c:2 * c + 2, :],
                             adj[:, 2 * c:2 * c + 2, :],
                             start=(c == 0), stop=(c == 3), perf_mode=DR)
            if comp:
                comp.pop(0)()
        for f in comp:
            f()
        lo = t * 512
        if relu_split:  # tail: release h in quarters so proj starts sooner
            nc.scalar.activation(h_new[:, lo:lo + 256], ps[:, 0:256], RELU,
                                 scale=S[i] / S[i + 1])
            nc.scalar.activation(h_new[:, lo + 256:lo + 512], ps[:, 256:512],
                                 RELU, scale=S[i] / S[i + 1])
        else:
            nc.scalar.activation(h_new[:, lo:lo + 512], ps[:], RELU,
                                 scale=S[i] / S[i + 1])

    # ---- schedule ------------------------------------------------------
    # 4-way graph interleave: units U(g, layer, t) in order g=0..3 per
    # t-half. Each unit carries one companion burst chosen so every
    # relu -> msg -> fp8-quantize chain has >=2 units (~1.7us) of slack
    # before its consumer, and the PE stream never thins: msg/proj/embed
    # matmuls ride between DoubleRow matmuls.
    emit_embed_t(0, 0)
    emit_embed_t(0, 1)
    emit_embed_t(1, 0)
    emit_embed_t(1, 1)
    emit_bias_bcast(0)
    for f in burst_msg(0, 0, 0) + burst_msg(0, 0, 1)             + burst_msg(1, 0, 0) + burst_msg(1, 0, 1):
        f()
    for w in range(2):  # dependency-free bridge: a late first-adj
        psw = psA.tile([HID, 512], FP32, tag="psA", name="psw2")
        nc.tensor.matmul(psw[:], warm_w[:], warm_m[:], start=True, stop=True)

    # layer 0 (prologue for graphs 2,3 rides the first units)
    emit_unit(0, 0, 0, [lambda: emit_embed_t(2, 0), lambda: emit_embed_t(2, 1),
                        lambda: emit_bias_bcast(1)])
    emit_unit(1, 0, 0, [lambda: emit_embed_t(3, 0), lambda: emit_embed_t(3, 1)]
              + burst_msg(2, 0, 0))
    for f in burst_msg(2, 0, 1) + burst_msg(3, 0, 0):
        f()
    emit_unit(2, 0, 0, burst_msg(3, 0, 1) + [lambda: emit_bias_bcast(2)])
    emit_unit(3, 0, 0, burst_msg(0, 1, 0))
    emit_unit(0, 0, 1, burst_msg(1, 1, 0))
    emit_unit(1, 0, 1, burst_msg(2, 1, 0))
    emit_unit(2, 0, 1, burst_msg(3, 1, 0))
    emit_unit(3, 0, 1, burst_msg(0, 1, 1))
    # layer 1
    emit_unit(0, 1, 0, burst_msg(1, 1, 1))
    emit_unit(1, 1, 0, burst_msg(2, 1, 1))
    emit_unit(2, 1, 0, burst_msg(3, 1, 1))
    emit_unit(3, 1, 0, burst_msg(0, 2, 0))
    emit_unit(0, 1, 1, burst_msg(1, 2, 0))
    emit_unit(1, 1, 1, burst_msg(2, 2, 0))
    emit_unit(2, 1, 1, burst_msg(3, 2, 0))
    emit_unit(3, 1, 1, burst_msg(0, 2, 1))
    # layer 2 (projection rides the msg slots)
    emit_unit(0, 2, 0, burst_msg(1, 2, 1))
    emit_unit(1, 2, 0, burst_msg(2, 2, 1))
    emit_unit(2, 2, 0, burst_msg(3, 2, 1))
    emit_unit(3, 2, 0, [lambda: emit_proj_half(0, 0)])
    emit_unit(0, 2, 1, [lambda: emit_proj_half(1, 0)])
    emit_unit(1, 2, 1, [lambda: emit_proj_half(2, 0)])
    emit_unit(2, 2, 1, [lambda: emit_proj_half(3, 0),
                        lambda: emit_proj_half(0, 1)])
    emit_unit(3, 2, 1, [lambda: emit_proj_half(1, 1)], relu_split=True)
    emit_proj_half(2, 1)
    poq = psO.tile([OUT, 512], FP32, tag="psO")
    emit_proj_q(3, 1, 0, poq)
    emit_proj_q(3, 1, 1, poq)


def build_nc():
    # Bacc (not raw Bass): its compile() runs generate_event_semaphores,
    # which splits multi-sem waits down to the 1-wait-per-instruction
    # hardware limit walrus enforces.
    nc = bacc.Bacc("TRN2", debug=False, num_devices=N_CORES, num_swdge_queues=2)
    adjT8 = nc.dram_tensor("adjT8", [BPC, NT, 128, NC8, 512], FP8,
                           kind="ExternalInput").ap()
    packA = nc.dram_tensor("packA", [64, PACKA_W], FP16,
                           kind="ExternalInput").ap()
    packB = nc.dram_tensor("packB", [128, PACKB_W], FP16,
                           kind="ExternalInput").ap()
    xT = nc.dram_tensor("xT", [BPC, IN_DIM, N], FP16, kind="ExternalInput").ap()
    out = nc.dram_tensor("out", [BPC, OUT, N], FP16, kind="ExternalOutput").ap()

    with tile.TileContext(nc) as tc, ExitStack() as ctx:
        _kernel_body(ctx, tc, out, adjT8, packA, packB, xT)
    nc.compile()
    return nc


def make_in_maps(node_features, adjacency_matrix, node_mask, W_embed, Wl, bl,
                 W_proj, b_proj):
    x = np.asarray(node_features, dtype=np.float32)
    adj = np.asarray(adjacency_matrix, dtype=np.float32)
    bl4 = np.tile(
        (np.asarray(bl, np.float64) * BLZ / np.array(S[:NUM_LAYERS])[:, None]),
        (1, 4))
    in_maps = []
    for c in range(N_CORES):
        sl = slice(c * BPC, (c + 1) * BPC)
        xTc = np.ascontiguousarray(x[sl].transpose(0, 2, 1)).astype(np.float16)
        pka = np.zeros((64, PACKA_W), np.float16)
        pka[:, PKA_WE:PKA_WE + HID] = np.asarray(W_embed, np.float16)
        pka[:, PKA_X0:PKA_X0 + N] = xTc[0]
        pkb = np.zeros((128, PACKB_W), np.float16)
        for i in range(NUM_LAYERS):
            pkb[:, PKB_WL + HID * i:PKB_WL + HID * (i + 1)] = np.asarray(
                Wl[i], np.float16)
            pkb[0, PKB_BL + 512 * i:PKB_BL + 512 * (i + 1)] = bl4[i].astype(
                np.float16)
        pkb[:, PKB_WP:PKB_WP + OUT] = np.asarray(W_proj, np.float16)
        pkb[0:64, PKB_X1:PKB_X1 + N] = xTc[1]
        # adjT8[bb, t, p, c, j] = adj[bb, t*512+j, c*128+p]
        a = adj[sl].reshape(BPC, NT, 512, NC8, 128).transpose(0, 1, 4, 3, 2)
        in_maps.append({
            "adjT8": np.ascontiguousarray(a).astype(ml_dtypes.float8_e4m3fn),
            "packA": pka,
            "packB": pkb,
            "xT": xTc,
        })
    return in_maps


_NC_CACHE = None


def get_nc():
    global _NC_CACHE
    if _NC_CACHE is None:
        _NC_CACHE = build_nc()
    return _NC_CACHE


def kernel(**inputs):
    nc = get_nc()
    in_maps = make_in_maps(**inputs)
    res = run_bass_kernel_spmd(nc, in_maps, list(range(N_CORES)))
    outs = [np.asarray(res.results[c]["out"]) for c in range(N_CORES)]
    oT = np.concatenate(outs, axis=0).astype(np.float32)  # [B, OUT, N]
    o = oT.transpose(0, 2, 1) * np.float32(S[NUM_LAYERS])
    o = o + np.asarray(inputs["b_proj"], np.float32)
    o = o * np.asarray(inputs["node_mask"], np.float32)
    return o


if __name__ == "__main__":
    rng = np.random.default_rng(0)
    ins = {
        "node_features": rng.standard_normal((B, N, IN_DIM), dtype=np.float32),
        "adjacency_matrix": rng.random((B, N, N), dtype=np.float32),
        "node_mask": np.ones((B, N, 1), np.float32),
        "W_embed": rng.standard_normal((IN_DIM, HID), dtype=np.float32) * 0.1,
        "Wl": rng.standard_normal((NUM_LAYERS, HID, HID), dtype=np.float32) * 0.08,
        "bl": rng.standard_normal((NUM_LAYERS, HID), dtype=np.float32) * 0.08,
        "W_proj": rng.standard_normal((HID, 2 * 32), dtype=np.float32) * 0.08,
        "b_proj": rng.standard_normal((2 * 32,), dtype=np.float32) * 0.08,
    }
    out = kernel(**ins)
    print("out", out.shape, out.dtype, float(np.abs(out).mean()))
